# revision 23
# baseline (speedup 1.0000x reference)
"""8-core TRN2 Bass kernel for the 6-layer GCN edge classifier — gather-only.

Per layer (nodes dst-sharded 8 ways, 12928 slots/core incl 428 phantoms):
- hw' = dinv * (xe @ W_l) computed from transposed xeT via per-block matmuls,
  staged node-major, AllGather -> replicated table [103424, 64] in HBM.
- Phase A: ELL-style gather. (dst, src-window) pairs sorted by degree, blocks
  of <=2048 pairs, round-r gathers the r-th edge of every still-active pair
  (prefix of the block). Rounds accumulate on DVE into an SBUF acc tile;
  at block end acc *= dinv[dst] and is flushed to ptable (HBM, 2x 32768-row
  halves so combine indices fit int16).
- Phase B: every dst has exactly 4 ptable partials (zero-padded). One gather
  pass + 2-column matmuls against a fixed S4 matrix reduce them, emitting
  aggT [128, 6464] transposed (partition = 64*u + dim, interleaved halves).
- BN stats via free-dim reduce + tiny AllReduce; fused affine + relu +
  residual on DVE, all 128 partitions wide.
- Head: y4 = [xe@fcW_top | xe@fcW_bot] per node -> table -> AllGather ->
  per-edge gathers; host adds halves + bias.

No dma_scatter_add anywhere (it dominated the previous version's runtime).
"""
import sys
import os

for _p in ("/opt/trn_rl_repo", "/root/.axon_site/_ro/trn_rl_repo"):
    if os.path.isdir(_p) and _p not in sys.path:
        sys.path.insert(0, _p)

import numpy as np
import concourse.bass as bass
import concourse.mybir as mybir
import concourse.tile as tile
from concourse import bacc
from concourse.bass_utils import run_bass_kernel_spmd
from concourse.masks import make_identity

# problem constants
N = 100000
E = 1600000
E_OUT = 400000
IN_DIM = 16
HID = 64
OUT_DIM = 2
L = 6
BN_EPS = 1e-5

NCORES = 8
NPC_REAL = 12500
NPC = 12928                # slots per core (428 phantoms)
COLS = NPC // 128          # 101 node blocks
TBLR = NCORES * NPC        # 103424 table rows
NWIN = 4
WINSZ = TBLR // NWIN       # 25856 (< 32768, int16-safe)
PT_HALF = 32768            # ptable half (window) size
CALLN = 4096               # max idx per gather call
BLKP = 2048                # max pairs per ELL block
HALFS = NPC // 2           # 6464 slots per partition-half

f32 = mybir.dt.float32
i16 = mybir.dt.int16

# slot <-> table-row permutation (from the hw'-block matmul layout):
# staging col j partition p holds slot S = 128j + 64*((p%64)//32) + 32*(p//64)
# + (p%32); bounce/table row r = 128j + p.
_P = np.arange(128)
_J = np.arange(COLS)
SLOT_OF_PJ = (128 * _J[None, :] + 64 * ((_P[:, None] % 64) // 32)
              + 32 * (_P[:, None] // 64) + (_P[:, None] % 32))   # [128, COLS]
ROW_OF_SLOT = np.empty(NPC, np.int64)
ROW_OF_SLOT[SLOT_OF_PJ.T.reshape(-1)] = (128 * np.repeat(np.arange(COLS), 128)
                                         + np.tile(np.arange(128), COLS))
assert np.array_equal(np.sort(SLOT_OF_PJ.reshape(-1)), np.arange(NPC))
ZERO_IDX = int(ROW_OF_SLOT[NPC_REAL])   # a phantom slot's row (< NPC < WINSZ)

_CACHE = {}


def _ceil(a, b):
    return -(-a // b)


def _wrap16(idx):
    """[n] int array (n % 16 == 0) -> [128, n//16] int16 for Ant DMA ops."""
    n = idx.shape[0]
    w = idx.reshape(n // 16, 16).T.astype(np.int16)
    return np.tile(w, (8, 1))


def _row_of_global(v):
    """global node id -> table row."""
    v = np.asarray(v, np.int64)
    return (v // NPC_REAL) * NPC + ROW_OF_SLOT[v % NPC_REAL]


def _preprocess(edge_index, edge_index_out):
    src = np.asarray(edge_index[0], dtype=np.int64)
    dst = np.asarray(edge_index[1], dtype=np.int64)
    deg = np.bincount(dst, minlength=N).astype(np.float64) + 1.0
    dinv_g = (1.0 / np.sqrt(deg)).astype(np.float32)

    # pass 1: per core, per section (w, P): pairs (dst, srcs) sorted by count
    # desc. secs[(w, P)][c] = (dsts, starts, counts_sorted_desc, rs)
    secs = {(w, P): [None] * NCORES for w in range(NWIN) for P in (0, 1)}
    for c in range(NCORES):
        m = (dst // NPC_REAL) == c
        s_g = np.concatenate([src[m], c * NPC_REAL + np.arange(NPC_REAL)])
        d_loc = np.concatenate([dst[m] - c * NPC_REAL, np.arange(NPC_REAL)])
        rows = _row_of_global(s_g)
        w_e = rows // WINSZ
        rb_e = rows - w_e * WINSZ
        p_half = (d_loc // 64) % 2
        order = np.lexsort((d_loc, p_half, w_e))
        w_e, rb_e, d2, p2 = w_e[order], rb_e[order], d_loc[order], p_half[order]
        for w in range(NWIN):
            for P in (0, 1):
                sel = (w_e == w) & (p2 == P)
                ds, rs = d2[sel], rb_e[sel]
                ud, starts, counts = np.unique(ds, return_index=True,
                                               return_counts=True)
                po = np.argsort(-counts, kind="stable")
                secs[(w, P)][c] = (ud[po], starts[po], counts[po], rs)

    # unified degree profile per section: U_j = max over cores of counts[j]
    profiles = {}
    for key, percore in secs.items():
        U_len = max(len(x[0]) for x in percore)
        U = np.zeros(U_len, np.int64)
        for (_, _, counts, _) in percore:
            U[:len(counts)] = np.maximum(U[:len(counts)], counts)
        profiles[key] = U

    # pass 2: shared block structure + per-core idx streams
    used = [0, 0]
    blocks = []                     # shared: P, rankbase, blkp, ncols
    streams = {w: [[] for _ in range(NCORES)] for w in range(NWIN)}
    segs_w = {w: [] for w in range(NWIN)}   # shared (cols, buid, first, last)
    ranks_of_dst = [np.full((NPC, 4), -1, np.int64) for _ in range(NCORES)]
    nrank_of_dst = [np.zeros(NPC, np.int64) for _ in range(NCORES)]
    dinvcols = [[] for _ in range(NCORES)]  # per block [128, ncols]

    for w in range(NWIN):
        for P in (0, 1):
            U = profiles[(w, P)]
            percore = secs[(w, P)]
            npairs = len(U)
            pos = 0
            while pos < npairs:
                take = min(BLKP, npairs - pos)
                blkp = _ceil(take, 128) * 128
                Ub = U[pos:pos + take]
                R = int(Ub[0]) if take else 0
                rankbase = used[P]
                buid = len(blocks)
                blocks.append(dict(P=P, rankbase=rankbase, blkp=blkp,
                                   ncols=blkp // 128))
                for c in range(NCORES):
                    ud, st, cnts, rs = percore[c]
                    nc_take = max(0, min(take, len(ud) - pos))
                    dsts = ud[pos:pos + nc_take]
                    nr = nrank_of_dst[c][dsts]
                    assert (nr < 4).all()
                    ranks_of_dst[c][dsts, nr] = (P * PT_HALF + rankbase
                                                 + np.arange(nc_take))
                    nrank_of_dst[c][dsts] = nr + 1
                    dcol = np.zeros(blkp, np.float32)
                    dcol[:nc_take] = dinv_g[c * NPC_REAL + dsts]
                    dinvcols[c].append(dcol.reshape(blkp // 128, 128).T)
                for r in range(R):
                    active = int(np.count_nonzero(Ub > r))
                    cols = _ceil(active, 128) if r > 0 else blkp // 128
                    segs_w[w].append((cols, buid, r == 0, r == R - 1))
                    for c in range(NCORES):
                        ud, st, cnts, rs = percore[c]
                        idx = np.full(cols * 128, ZERO_IDX, np.int64)
                        na = max(0, min(active, len(ud) - pos))
                        if na:
                            cb = cnts[pos:pos + na]
                            act = cb > r
                            idx[:na][act] = rs[st[pos:pos + na][act] + r]
                        streams[w][c].append(idx)
                used[P] += blkp
                pos += take
    assert used[0] + 128 <= PT_HALF and used[1] + 128 <= PT_HALF, used
    zero_rank = [used[0], used[1]]

    # slice window streams into calls (<= CALLN idx), shared structure
    acalls = []
    aidx = [[] for _ in range(NCORES)]
    for w in range(NWIN):
        rsegs = segs_w[w]
        if not rsegs:
            continue
        ri = 0
        roff = 0
        while ri < len(rsegs):
            n_cols = 0
            segs = []
            marks = []
            while ri < len(rsegs) and n_cols < CALLN // 128:
                cols, buid, first, last = rsegs[ri]
                avail = cols - roff
                takec = min(avail, CALLN // 128 - n_cols)
                segs.append((n_cols, takec, buid, roff, first,
                             last and (roff + takec == cols)))
                marks.append((ri, roff, takec))
                n_cols += takec
                roff += takec
                if roff == cols:
                    ri += 1
                    roff = 0
            acalls.append((w, n_cols * 128, segs))
            for c in range(NCORES):
                for (rj, ro, tk) in marks:
                    aidx[c].append(streams[w][c][rj][ro * 128:(ro + tk) * 128])
    aidx = [np.concatenate(a) for a in aidx]

    # phase B: per ptable half, slots in s-order, 4 ranks each (shared calls)
    bcalls = []
    bidx = [[] for _ in range(NCORES)]
    for P in (0, 1):
        slots = np.arange(NPC)
        slots = slots[((slots // 64) % 2) == P]
        idx_c = []
        for c in range(NCORES):
            ranks = ranks_of_dst[c][slots]
            zr = P * PT_HALF + zero_rank[P]
            ranks = np.where(ranks < 0, zr, ranks)
            ic = (ranks - P * PT_HALF).reshape(-1)
            assert ic.min() >= 0 and ic.max() < PT_HALF
            idx_c.append(ic)
        total = len(idx_c[0])
        off = 0
        while off < total:
            n = min(CALLN, total - off)
            chunks = []
            coff = 0
            while coff < n:
                cn = min(2048, n - coff)
                nq = cn // 4 // 64
                assert nq * 256 == cn, (cn, nq)
                a0 = (off + coff) // 4 // 64
                chunks.append((coff // 128, nq, a0))
                coff += cn
            bcalls.append((P, n, chunks))
            for c in range(NCORES):
                bidx[c].append(idx_c[c][off:off + n])
            off += n
    bidx = [np.concatenate(b) for b in bidx]

    plan = dict(acalls=acalls, bcalls=bcalls, blocks=blocks,
                zero_rank=zero_rank)
    cores = [dict(aidx=aidx[c], bidx=bidx[c], dinvcols=dinvcols[c])
             for c in range(NCORES)]

    EPC = E_OUT // NCORES
    fcalls, fpacked, fslotmap = _head_plan(edge_index_out, EPC)
    return dinv_g, plan, cores, fcalls, fpacked, fslotmap


def _head_plan(edge_index_out, EPC):
    fg = {}
    for c in range(NCORES):
        es = np.asarray(edge_index_out[0][c * EPC:(c + 1) * EPC], np.int64)
        ed = np.asarray(edge_index_out[1][c * EPC:(c + 1) * EPC], np.int64)
        allpos = np.concatenate([_row_of_global(es), _row_of_global(ed)])
        half = np.concatenate([np.zeros(EPC, np.int64), np.ones(EPC, np.int64)])
        eid = np.concatenate([np.arange(EPC), np.arange(EPC)])
        w = allpos // WINSZ
        key = half * NWIN + w
        order = np.lexsort((allpos, key))
        allpos, eid, half, key = allpos[order], eid[order], half[order], key[order]
        uk, starts = np.unique(key, return_index=True)
        starts = list(starts) + [len(allpos)]
        for i, k in enumerate(uk):
            fg.setdefault(int(k), [None] * NCORES)
            fg[int(k)][c] = (allpos[starts[i]:starts[i + 1]],
                             eid[starts[i]:starts[i + 1]],
                             half[starts[i]:starts[i + 1]])
    fcalls = []
    fpacked = [[] for _ in range(NCORES)]
    fslotmap = [[] for _ in range(NCORES)]
    CH = 2048
    for key in sorted(fg):
        w = key % NWIN
        base = w * WINSZ
        percore = fg[key]
        nmax = max((len(x[0]) if x is not None else 0) for x in percore)
        if nmax == 0:
            continue
        for ci in range(_ceil(nmax, CH)):
            size = min(CH, nmax - ci * CH)
            n_pad = _ceil(size, 16) * 16
            off = ci * CH
            for c in range(NCORES):
                ap_, eid_, half_ = percore[c] if percore[c] is not None else (
                    np.empty(0, np.int64), np.empty(0, np.int64),
                    np.empty(0, np.int64))
                spc = ap_[off:off + size]
                ei = eid_[off:off + size]
                hf = half_[off:off + size]
                npad = n_pad - len(spc)
                g = np.concatenate([spc - base, np.zeros(npad, np.int64)])
                assert g.min() >= 0 and g.max() < WINSZ
                fpacked[c].append(g)
                fslotmap[c].append((ei, hf))
            fcalls.append((w, n_pad))
    return fcalls, fpacked, fslotmap


def _build_program(plan, fcalls, repeat=1, body=True, no_phase_a=False,
                   no_phase_b=False):
    acalls, bcalls, blocks = plan["acalls"], plan["bcalls"], plan["blocks"]
    nc = bacc.Bacc("TRN2", target_bir_lowering=False, debug=False,
                   num_devices=NCORES, num_swdge_queues=4)
    GCA = sum(n // 16 for _, n, _ in acalls)
    GCB = sum(n // 16 for _, n, _ in bcalls)
    FC = sum(n // 16 for _, n in fcalls)
    TOTS = sum(_ceil(n, 128) * 128 for _, n in fcalls)
    NBLK = len(blocks)

    xT_in = nc.dram_tensor("xT", [IN_DIM, 2, HALFS], f32, kind="ExternalInput")
    dinvnm_in = nc.dram_tensor("dinvnm", [128, COLS], f32, kind="ExternalInput")
    bembT_in = nc.dram_tensor("bembT", [128, 1], f32, kind="ExternalInput")
    wemb_in = nc.dram_tensor("wemb", [IN_DIM, HID], f32, kind="ExternalInput")
    convw_in = nc.dram_tensor("convw", [128, L * HID], f32, kind="ExternalInput")
    bn_in = nc.dram_tensor("bn", [L, 128], f32, kind="ExternalInput")
    fcw_in = nc.dram_tensor("fcw", [128, 4], f32, kind="ExternalInput")
    dinvblk_in = nc.dram_tensor("dinvblk", [128, 16 * NBLK], f32,
                                kind="ExternalInput")
    gidxA_in = nc.dram_tensor("gidxA", [128, GCA], i16, kind="ExternalInput")
    gidxB_in = nc.dram_tensor("gidxB", [128, GCB], i16, kind="ExternalInput")
    fidx_in = nc.dram_tensor("fidx", [128, FC], i16, kind="ExternalInput")
    yout = nc.dram_tensor("yout", [TOTS, 4], f32, kind="ExternalOutput")

    with tile.TileContext(nc) as tc:
        with (
            tc.tile_pool(name="const", bufs=1) as cp,
            tc.tile_pool(name="xtp", bufs=1) as xtp,
            tc.tile_pool(name="big", bufs=1) as bp,
            tc.tile_pool(name="accp", bufs=2) as ap_,
            tc.tile_pool(name="msg", bufs=3) as mp,
            tc.tile_pool(name="work", bufs=2) as wp,
            tc.tile_pool(name="psA", bufs=2, space="PSUM") as psA,
            tc.tile_pool(name="psB", bufs=2, space="PSUM") as psB,
            tc.tile_pool(name="psT", bufs=1, space="PSUM") as psT,
            tc.tile_pool(name="dram", bufs=1, space="DRAM") as dp,
        ):
            # ---- DRAM internals
            bounce = dp.tile([NPC, HID], f32)
            table = dp.tile([TBLR, HID], f32)
            ptable = dp.tile([2 * PT_HALF, HID], f32)
            arb_in = dp.tile([2, HID], f32)
            arb_out = dp.tile([2, HID], f32)

            # ---- constants / inputs to SBUF
            id128 = cp.tile([128, 128], f32)
            make_identity(nc, id128[:])
            dinvnm = cp.tile([128, COLS], f32)
            nc.sync.dma_start(out=dinvnm[:], in_=dinvnm_in[:])
            bembT = cp.tile([128, 1], f32)
            nc.sync.dma_start(out=bembT[:], in_=bembT_in[:])
            wemb = cp.tile([IN_DIM, HID], f32)
            nc.sync.dma_start(out=wemb[:], in_=wemb_in[:])
            convw = cp.tile([128, L * HID], f32)
            nc.sync.dma_start(out=convw[:], in_=convw_in[:])
            fcw = cp.tile([128, 4], f32)
            nc.sync.dma_start(out=fcw[:], in_=fcw_in[:])
            dinvblk = cp.tile([128, 16 * NBLK], f32)
            nc.sync.dma_start(out=dinvblk[:], in_=dinvblk_in[:])
            gidxA = cp.tile([128, GCA], i16)
            nc.sync.dma_start(out=gidxA[:], in_=gidxA_in[:])
            gidxB = cp.tile([128, GCB], i16)
            nc.sync.dma_start(out=gidxB[:], in_=gidxB_in[:])
            S4 = cp.tile([128, 32], f32)
            # S4[p, g] = (p//4 == g): sum id128 columns in groups of 4
            id_v = id128[:].rearrange("p (g k) -> p g k", k=4)
            nc.vector.tensor_reduce(out=S4[:], in_=id_v,
                                    axis=mybir.AxisListType.X,
                                    op=mybir.AluOpType.add)

            _regs = {}

            def reg_of(n):
                if n not in _regs:
                    _regs[n] = nc.gpsimd.to_reg(n)
                return _regs[n]

            xeT = bp.tile([128, HALFS], f32)
            aggT = bp.tile([128, HALFS], f32)
            staging = bp.tile([128, COLS, HID], f32)
            aggT_v = aggT[:].rearrange("p (a b v) -> p a b v", b=2, v=32)

            # ---- pre-zero the ptable zero-rows (one per half), once
            ztile = wp.tile([128, 1, HID], f32, tag="ztile")
            nc.vector.memset(ztile[:], 0.0)
            for P in (0, 1):
                zr = P * PT_HALF + plan["zero_rank"][P]
                nc.sync.dma_start(
                    out=ptable[zr:zr + 128, :].rearrange("(c p) k -> p c k",
                                                         p=128),
                    in_=ztile[:, :, :])

            # ---- embed: xeT[64u+d, f] = sum_i x[i, slot] wemb[i, d] + bemb[d]
            EW = 256
            for u in (0, 1):
                xth = xtp.tile([IN_DIM, HALFS], f32, tag="xth")
                nc.sync.dma_start(out=xth[:], in_=xT_in[:, u, :])
                for fb in (range(0, HALFS, EW) if body else []):
                    wd = min(EW, HALFS - fb)
                    ps = psB.tile([128, EW], f32, tag="pb")
                    nc.tensor.matmul(out=ps[64 * u:64 * u + 64, :wd],
                                     lhsT=wemb[:], rhs=xth[:, fb:fb + wd],
                                     start=True, stop=True)
                    nc.vector.tensor_scalar_add(
                        out=xeT[64 * u:64 * u + 64, fb:fb + wd],
                        in0=ps[64 * u:64 * u + 64, :wd],
                        scalar1=bembT[64 * u:64 * u + 64, :])

            # ---- layers
            for l in ([li for _ in range(repeat) for li in range(L)]
                      if body else []):
                _emit_hw_blocks(nc, psA, xeT, convw, l * HID, staging, dinvnm)
                nc.sync.dma_start(
                    out=bounce[:].rearrange("(c p) k -> p c k", p=128),
                    in_=staging[:, :, :])
                nc.gpsimd.collective_compute(
                    "AllGather", mybir.AluOpType.bypass,
                    replica_groups=[list(range(NCORES))],
                    ins=[bounce[:].opt()], outs=[table[:].opt()])

                # ---- phase A
                if not no_phase_a:
                    off = 0
                    accs = {}
                    for k, (w, n, segs) in enumerate(acalls):
                        cc = n // 128
                        msg = mp.tile([128, CALLN // 128, HID], f32, tag="msg")
                        nc.gpsimd.dma_gather(
                            out_ap=msg[:, :cc, :],
                            in_ap=table[w * WINSZ:(w + 1) * WINSZ, :],
                            idxs_ap=gidxA[:, off:off + n // 16],
                            num_idxs=n, num_idxs_reg=reg_of(n), elem_size=HID,
                            single_packet=False, queue_num=k % 4)
                        off += n // 16
                        for (mcol, ncols, buid, acol, first, flush) in segs:
                            b = blocks[buid]
                            if first and acol == 0:
                                acc = ap_.tile([128, 16, HID], f32, tag="acc")
                                accs[buid] = acc
                            acc = accs[buid]
                            src_ap = msg[:, mcol:mcol + ncols, :]
                            dst_ap = acc[:, acol:acol + ncols, :]
                            if first:
                                nc.vector.tensor_copy(out=dst_ap, in_=src_ap)
                            else:
                                nc.vector.tensor_tensor(
                                    out=dst_ap, in0=dst_ap, in1=src_ap,
                                    op=mybir.AluOpType.add)
                            if flush:
                                nco = b["ncols"]
                                dv = dinvblk[:, 16 * buid:16 * buid + nco]
                                dv = dv.rearrange("p (c o) -> p c o", o=1)
                                nc.vector.tensor_tensor(
                                    out=acc[:, :nco, :], in0=acc[:, :nco, :],
                                    in1=dv.to_broadcast([128, nco, HID]),
                                    op=mybir.AluOpType.mult)
                                rb = b["P"] * PT_HALF + b["rankbase"]
                                nc.sync.dma_start(
                                    out=ptable[rb:rb + b["blkp"], :].rearrange(
                                        "(c p) k -> p c k", p=128),
                                    in_=acc[:, :nco, :])
                                del accs[buid]

                # ---- phase B
                if not no_phase_b:
                    off = 0
                    for k, (P, n, chunks) in enumerate(bcalls):
                        cc = n // 128
                        msg = mp.tile([128, CALLN // 128, HID], f32, tag="msg")
                        nc.gpsimd.dma_gather(
                            out_ap=msg[:, :cc, :],
                            in_ap=ptable[P * PT_HALF:(P + 1) * PT_HALF, :],
                            idxs_ap=gidxB[:, off:off + n // 16],
                            num_idxs=n, num_idxs_reg=reg_of(n), elem_size=HID,
                            single_packet=False, queue_num=k % 4)
                        off += n // 16
                        for (ccol, nq, a0) in chunks:
                            ps = psB.tile([128, 256], f32, tag="pb")
                            for i in range(nq):
                                lt = msg[:, ccol + 2 * i:ccol + 2 * i + 2, :]
                                nc.tensor.matmul(
                                    out=ps[:, 32 * i:32 * i + 32],
                                    lhsT=lt.rearrange("p a k -> p (a k)"),
                                    rhs=S4[:], start=True, stop=True)
                            nc.vector.tensor_copy(
                                out=aggT_v[:, a0:a0 + nq, P:P + 1, :],
                                in_=ps[:].rearrange("p (i o g) -> p i o g",
                                                    o=1, g=32)[:, :nq, :, :])

                # ---- BN stats (free-dim reduces on transposed agg)
                stats2 = wp.tile([128, 2], f32, tag="stats2")
                nc.vector.tensor_reduce(
                    out=stats2[:, 0:1],
                    in_=aggT[:].rearrange("p (o f) -> p o f", o=1),
                    axis=mybir.AxisListType.X, op=mybir.AluOpType.add)
                sqv = staging[:].rearrange("p c k -> p (c k)")
                nc.vector.tensor_tensor(out=sqv, in0=aggT[:], in1=aggT[:],
                                        op=mybir.AluOpType.mult)
                nc.vector.tensor_reduce(
                    out=stats2[:, 1:2],
                    in_=staging[:, :, :],
                    axis=mybir.AxisListType.XY, op=mybir.AluOpType.add)
                trp = psT.tile([128, 128], f32, tag="tr")
                psS = trp[0:2, :]
                nc.tensor.transpose(out=psS, in_=stats2[:], identity=id128[:])
                st2 = wp.tile([2, 128], f32, tag="st2")
                nc.vector.tensor_copy(out=st2[:], in_=psS)
                hc = wp.tile([2, HID], f32, tag="hc")
                nc.vector.tensor_tensor(out=hc[:], in0=st2[:, 0:HID],
                                        in1=st2[:, HID:128],
                                        op=mybir.AluOpType.add)
                nc.sync.dma_start(out=arb_in[:], in_=hc[:])
                nc.gpsimd.collective_compute(
                    "AllReduce", mybir.AluOpType.add,
                    replica_groups=[list(range(NCORES))],
                    ins=[arb_in[:].opt()], outs=[arb_out[:].opt()])
                gs = wp.tile([2, HID], f32, tag="gs")
                nc.sync.dma_start(out=gs[:], in_=arb_out[:])
                trq = psT.tile([128, 128], f32, tag="tr")
                psQ = trq[0:1, 0:HID]
                nc.tensor.matmul(out=psQ, lhsT=id128[0:2, 1:2], rhs=gs[:],
                                 start=True, stop=True)
                sq_row = wp.tile([1, HID], f32, tag="sqrow")
                nc.vector.tensor_scalar_mul(out=sq_row[:], in0=psQ,
                                            scalar1=1.0 / N)
                mean = wp.tile([1, HID], f32, tag="mean")
                nc.vector.tensor_scalar_mul(out=mean[:], in0=gs[0:1, :],
                                            scalar1=1.0 / N)
                var = wp.tile([1, HID], f32, tag="var")
                nc.vector.tensor_tensor(out=var[:], in0=mean[:], in1=mean[:],
                                        op=mybir.AluOpType.mult)
                nc.vector.tensor_tensor(out=var[:], in0=sq_row[:], in1=var[:],
                                        op=mybir.AluOpType.subtract)
                nc.vector.tensor_scalar_add(out=var[:], in0=var[:],
                                            scalar1=float(BN_EPS))
                sd = wp.tile([1, HID], f32, tag="sd")
                nc.scalar.activation(out=sd[:], in_=var[:],
                                     func=mybir.ActivationFunctionType.Sqrt)
                rs = wp.tile([1, HID], f32, tag="rs")
                nc.vector.reciprocal(out=rs[:], in_=sd[:])
                bnl = wp.tile([1, 128], f32, tag="bnl")
                nc.sync.dma_start(out=bnl[:], in_=bn_in[l:l + 1, :])
                srow = wp.tile([1, 128], f32, tag="srow")
                trow = wp.tile([1, 128], f32, tag="trow")
                nc.vector.tensor_tensor(out=srow[:, 0:HID], in0=bnl[:, 0:HID],
                                        in1=rs[:], op=mybir.AluOpType.mult)
                nc.vector.tensor_copy(out=srow[:, HID:128], in_=srow[:, 0:HID])
                tmp = wp.tile([1, HID], f32, tag="tmp")
                nc.vector.tensor_tensor(out=tmp[:], in0=mean[:],
                                        in1=srow[:, 0:HID],
                                        op=mybir.AluOpType.mult)
                nc.vector.tensor_tensor(out=trow[:, 0:HID], in0=bnl[:, HID:128],
                                        in1=tmp[:], op=mybir.AluOpType.subtract)
                nc.vector.tensor_copy(out=trow[:, HID:128], in_=trow[:, 0:HID])
                trc = psT.tile([128, 128], f32, tag="tr")
                nc.tensor.transpose(out=trc[:, 0:1], in_=srow[:],
                                    identity=id128[0:1, 0:1])
                nc.tensor.transpose(out=trc[:, 1:2], in_=trow[:],
                                    identity=id128[0:1, 0:1])
                stc = wp.tile([128, 2], f32, tag="stc")
                nc.vector.tensor_copy(out=stc[:], in_=trc[:, 0:2])

                # apply: xeT += relu(aggT * s + t)
                nc.vector.tensor_scalar(
                    out=aggT[:], in0=aggT[:], scalar1=stc[:, 0:1],
                    scalar2=stc[:, 1:2], op0=mybir.AluOpType.mult,
                    op1=mybir.AluOpType.add)
                nc.vector.tensor_scalar_max(out=aggT[:], in0=aggT[:],
                                            scalar1=0.0)
                nc.vector.tensor_tensor(out=xeT[:], in0=xeT[:], in1=aggT[:],
                                        op=mybir.AluOpType.add)

            # ---- head: y4 per node -> table -> AllGather -> edge gathers
            nc.vector.memset(staging[:], 0.0)
            for j in range(COLS if body else 0):
                ps = psA.tile([128, HID], f32, tag="hw")
                _emit_block_mm(nc, ps, xeT, fcw, 0, j, out_w=4)
                nc.vector.tensor_copy(out=staging[:, j, 0:4], in_=ps[:, 0:4])
            nc.sync.dma_start(
                out=bounce[:].rearrange("(c p) k -> p c k", p=128),
                in_=staging[:, :, :])
            nc.gpsimd.collective_compute(
                "AllGather", mybir.AluOpType.bypass,
                replica_groups=[list(range(NCORES))],
                ins=[bounce[:].opt()], outs=[table[:].opt()])

            assert FC <= GCA, (FC, GCA)
            nc.sync.dma_start(out=gidxA[:, :FC], in_=fidx_in[:])
            off = 0
            soff = 0
            for k, (w, n_pad) in enumerate(fcalls if body else []):
                cc = _ceil(n_pad, 128)
                msg = mp.tile([128, CALLN // 128, HID], f32, tag="msg")
                nc.gpsimd.dma_gather(
                    out_ap=msg[:, :cc, :],
                    in_ap=table[w * WINSZ:(w + 1) * WINSZ, :],
                    idxs_ap=gidxA[:, off:off + n_pad // 16],
                    num_idxs=n_pad, num_idxs_reg=reg_of(n_pad), elem_size=HID,
                    single_packet=False, queue_num=k % 4)
                yo = mp.tile([128, CALLN // 128, 4], f32, tag="yo")
                nc.vector.tensor_copy(out=yo[:, :cc, :], in_=msg[:, :cc, 0:4])
                nc.sync.dma_start(
                    out=yout[soff:soff + cc * 128, :].rearrange(
                        "(p c) k -> p c k", p=128),
                    in_=yo[:, :cc, :])
                off += n_pad // 16
                soff += cc * 128
            if not body:
                yo0 = mp.tile([128, CALLN // 128, 4], f32, tag="yo")
                nc.vector.memset(yo0[:], 0.0)
                nc.sync.dma_start(
                    out=yout[0:(CALLN // 128) * 128, :].rearrange(
                        "(p c) k -> p c k", p=128),
                    in_=yo0[:, :, :])
    nc.compile()
    return nc, TOTS


def _emit_hw_blocks(nc, psA, xeT, convw, cbase, staging, dinvnm):
    for j in range(COLS):
        ps = psA.tile([128, HID], f32, tag="hw")
        _emit_block_mm(nc, ps, xeT, convw, cbase, j, out_w=HID)
        nc.vector.tensor_scalar_mul(out=staging[:, j, :], in0=ps[:],
                                    scalar1=dinvnm[:, j:j + 1])


def _emit_block_mm(nc, ps, xeT, rhs_tile, cbase, j, out_w):
    """ps[p, :] = (xe @ W) for node block j; rhs_tile is [128, *] with both
    halves stacked (rows 0:64 and 64:128 hold the same weights)."""
    fb = 64 * j
    nc.tensor.matmul(out=ps[0:64, :out_w], lhsT=xeT[0:64, fb:fb + 64],
                     rhs=rhs_tile[0:64, cbase:cbase + out_w],
                     start=True, stop=True)
    nc.tensor.matmul(out=ps[64:128, :out_w], lhsT=xeT[64:128, fb:fb + 64],
                     rhs=rhs_tile[64:128, cbase:cbase + out_w],
                     start=True, stop=True)


def _pack_inputs(inputs, dinv_g, cores, plan, fpacked):
    x = np.asarray(inputs["x"], np.float32)
    W_emb = np.asarray(inputs["W_emb"], np.float32)
    b_emb = np.asarray(inputs["b_emb"], np.float32)
    conv_W = np.asarray(inputs["conv_W"], np.float32)
    bn_gamma = np.asarray(inputs["bn_gamma"], np.float32)
    bn_beta = np.asarray(inputs["bn_beta"], np.float32)
    fc_W = np.asarray(inputs["fc_W"], np.float32)

    convw = np.transpose(conv_W, (1, 0, 2)).reshape(HID, L * HID)
    convw2 = np.concatenate([convw, convw], axis=0)          # [128, L*64]
    fcw_cat = np.concatenate([fc_W[:HID], fc_W[HID:]], axis=1)  # [64, 4]
    fcw2 = np.concatenate([fcw_cat, fcw_cat], axis=0)        # [128, 4]
    bn_cat = np.concatenate([bn_gamma, bn_beta], axis=1)     # [L, 128]
    bembT = np.tile(b_emb, 2).reshape(128, 1)

    NBLK = len(plan["blocks"])
    in_maps = []
    for c in range(NCORES):
        core = cores[c]
        xs = np.zeros((NPC, IN_DIM), np.float32)
        xs[:NPC_REAL] = x[c * NPC_REAL:(c + 1) * NPC_REAL]
        dv = np.zeros(NPC, np.float32)
        dv[:NPC_REAL] = dinv_g[c * NPC_REAL:(c + 1) * NPC_REAL]
        # xT[i, u, f] = x[slot(u,f), i], slot = 64*(f//32) + 32*u + f%32
        f = np.arange(HALFS)
        slot_u0 = 64 * (f // 32) + (f % 32)
        slot_u1 = slot_u0 + 32
        xT = np.stack([xs[slot_u0].T, xs[slot_u1].T], axis=1)  # [16, 2, HALFS]
        dinvnm = dv[SLOT_OF_PJ]                                # [128, COLS]
        dinvblk = np.zeros((128, 16 * NBLK), np.float32)
        for buid, arr in enumerate(core["dinvcols"]):
            dinvblk[:, 16 * buid:16 * buid + arr.shape[1]] = arr
        in_maps.append(dict(
            xT=np.ascontiguousarray(xT),
            dinvnm=np.ascontiguousarray(dinvnm),
            bembT=bembT, wemb=W_emb, convw=np.ascontiguousarray(convw2),
            bn=bn_cat, fcw=fcw2,
            dinvblk=dinvblk,
            gidxA=_wrap16(core["aidx"]),
            gidxB=_wrap16(core["bidx"]),
            fidx=np.concatenate([_wrap16(a) for a in fpacked[c]], axis=1),
        ))
    return in_maps


def _prepare(inputs):
    edge_index = np.asarray(inputs["edge_index"])
    edge_index_out = np.asarray(inputs["edge_index_out"])
    key = hash((edge_index[0, :50].tobytes(), edge_index_out[0, :50].tobytes()))
    if key in _CACHE:
        return _CACHE[key]
    dinv_g, plan, cores, fcalls, fpacked, fslotmap = _preprocess(
        edge_index, edge_index_out)
    nc, TOTS = _build_program(plan, fcalls)
    _CACHE[key] = (dinv_g, plan, cores, fcalls, fpacked, fslotmap, nc, TOTS)
    return _CACHE[key]


def kernel(x, edge_index, edge_index_out, W_emb, b_emb, conv_W, conv_b,
           bn_gamma, bn_beta, fc_W, fc_b):
    (dinv_g, plan, cores, fcalls, fpacked, fslotmap, nc, TOTS) = _prepare(
        dict(edge_index=edge_index, edge_index_out=edge_index_out))
    in_maps = _pack_inputs(
        dict(x=x, W_emb=W_emb, b_emb=b_emb, conv_W=conv_W, bn_gamma=bn_gamma,
             bn_beta=bn_beta, fc_W=fc_W),
        dinv_g, cores, plan, fpacked)
    res = run_bass_kernel_spmd(nc, in_maps, core_ids=list(range(NCORES)))

    EPC = E_OUT // NCORES
    out = np.zeros((E_OUT, OUT_DIM), np.float32)
    fc_b = np.asarray(fc_b, np.float32)
    for c in range(NCORES):
        y = res.results[c]["yout"]
        soff = 0
        for k, (w, n_pad) in enumerate(fcalls):
            cc = _ceil(n_pad, 128)
            eid, half = fslotmap[c][k]
            nreal = len(eid)
            i = np.arange(nreal)
            rows = soff + (i % 128) * cc + i // 128
            vals = y[rows]
            sel_src = half == 0
            out[c * EPC + eid[sel_src], :] += vals[sel_src][:, 0:2]
            out[c * EPC + eid[~sel_src], :] += vals[~sel_src][:, 2:4]
            soff += cc * 128
    out += fc_b[None, :]
    return out


# revision 29
# speedup vs baseline: 1.0122x; 1.0122x over previous
"""8-core TRN2 Bass kernel for the 6-layer GCN edge classifier — gather-only.

Per layer (nodes dst-sharded 8 ways, 12928 slots/core incl 428 phantoms):
- hw' = dinv * (xe @ W_l) computed from transposed xeT via per-block matmuls,
  staged node-major, AllGather -> replicated table [103424, 64] in HBM.
- Phase A: ELL-style gather. (dst, src-window) pairs sorted by degree, blocks
  of <=2048 pairs, round-r gathers the r-th edge of every still-active pair
  (prefix of the block). Rounds accumulate on DVE into an SBUF acc tile;
  at block end acc *= dinv[dst] and is flushed to ptable (HBM, 2x 32768-row
  halves so combine indices fit int16).
- Phase B: every dst has exactly 4 ptable partials (zero-padded). One gather
  pass + 2-column matmuls against a fixed S4 matrix reduce them, emitting
  aggT [128, 6464] transposed (partition = 64*u + dim, interleaved halves).
- BN stats via free-dim reduce + tiny AllReduce; fused affine + relu +
  residual on DVE, all 128 partitions wide.
- Head: y4 = [xe@fcW_top | xe@fcW_bot] per node -> table -> AllGather ->
  per-edge gathers; host adds halves + bias.

No dma_scatter_add anywhere (it dominated the previous version's runtime).
"""
import sys
import os

for _p in ("/opt/trn_rl_repo", "/root/.axon_site/_ro/trn_rl_repo"):
    if os.path.isdir(_p) and _p not in sys.path:
        sys.path.insert(0, _p)

import numpy as np
import concourse.bass as bass
import concourse.mybir as mybir
import concourse.tile as tile
from concourse import bacc
from concourse.bass_utils import run_bass_kernel_spmd
from concourse.masks import make_identity

# problem constants
N = 100000
E = 1600000
E_OUT = 400000
IN_DIM = 16
HID = 64
OUT_DIM = 2
L = 6
BN_EPS = 1e-5

NCORES = 8
NPC_REAL = 12500
NPC = 12928                # slots per core (428 phantoms)
COLS = NPC // 128          # 101 node blocks
TBLR = NCORES * NPC        # 103424 table rows
NWIN = 4
WINSZ = TBLR // NWIN       # 25856 (< 32768, int16-safe)
PT_HALF = 32768            # ptable half (window) size
CALLN = 4096               # max idx per gather call
BLKP = 2048                # max pairs per ELL block
HALFS = NPC // 2           # 6464 slots per partition-half

f32 = mybir.dt.float32
i16 = mybir.dt.int16

# slot <-> table-row permutation (from the hw'-block matmul layout):
# staging col j partition p holds slot S = 128j + 64*((p%64)//32) + 32*(p//64)
# + (p%32); bounce/table row r = p*COLS + j (partition-major so each SBUF
# partition's DMA stream is one contiguous block -> few fat descriptors).
_P = np.arange(128)
_J = np.arange(COLS)
SLOT_OF_PJ = (128 * _J[None, :] + 64 * ((_P[:, None] % 64) // 32)
              + 32 * (_P[:, None] // 64) + (_P[:, None] % 32))   # [128, COLS]
ROW_OF_SLOT = np.empty(NPC, np.int64)
ROW_OF_SLOT[SLOT_OF_PJ.reshape(-1)] = (COLS * np.repeat(np.arange(128), COLS)
                                       + np.tile(np.arange(COLS), 128))
assert np.array_equal(np.sort(SLOT_OF_PJ.reshape(-1)), np.arange(NPC))
ZERO_IDX = int(ROW_OF_SLOT[NPC_REAL])   # a phantom slot's row (< NPC < WINSZ)

_CACHE = {}


def _ceil(a, b):
    return -(-a // b)


def _wrap16(idx):
    """[n] int array (n % 16 == 0) -> [128, n//16] int16 for Ant DMA ops."""
    n = idx.shape[0]
    w = idx.reshape(n // 16, 16).T.astype(np.int16)
    return np.tile(w, (8, 1))


def _row_of_global(v):
    """global node id -> table row."""
    v = np.asarray(v, np.int64)
    return (v // NPC_REAL) * NPC + ROW_OF_SLOT[v % NPC_REAL]


def _preprocess(edge_index, edge_index_out):
    src = np.asarray(edge_index[0], dtype=np.int64)
    dst = np.asarray(edge_index[1], dtype=np.int64)
    deg = np.bincount(dst, minlength=N).astype(np.float64) + 1.0
    dinv_g = (1.0 / np.sqrt(deg)).astype(np.float32)

    # pass 1: per core, per section (w, P): pairs (dst, srcs) sorted by count
    # desc. secs[(w, P)][c] = (dsts, starts, counts_sorted_desc, rs)
    secs = {(w, P): [None] * NCORES for w in range(NWIN) for P in (0, 1)}
    for c in range(NCORES):
        m = (dst // NPC_REAL) == c
        s_g = np.concatenate([src[m], c * NPC_REAL + np.arange(NPC_REAL)])
        d_loc = np.concatenate([dst[m] - c * NPC_REAL, np.arange(NPC_REAL)])
        rows = _row_of_global(s_g)
        w_e = rows // WINSZ
        rb_e = rows - w_e * WINSZ
        p_half = (d_loc // 64) % 2
        order = np.lexsort((d_loc, p_half, w_e))
        w_e, rb_e, d2, p2 = w_e[order], rb_e[order], d_loc[order], p_half[order]
        for w in range(NWIN):
            for P in (0, 1):
                sel = (w_e == w) & (p2 == P)
                ds, rs = d2[sel], rb_e[sel]
                ud, starts, counts = np.unique(ds, return_index=True,
                                               return_counts=True)
                po = np.argsort(-counts, kind="stable")
                secs[(w, P)][c] = (ud[po], starts[po], counts[po], rs)

    # unified degree profile per section: U_j = max over cores of counts[j]
    profiles = {}
    for key, percore in secs.items():
        U_len = max(len(x[0]) for x in percore)
        U = np.zeros(U_len, np.int64)
        for (_, _, counts, _) in percore:
            U[:len(counts)] = np.maximum(U[:len(counts)], counts)
        profiles[key] = U

    # pass 2: shared block structure + per-core idx streams
    used = [0, 0]
    blocks = []                     # shared: P, rankbase, blkp, ncols
    streams = {w: [[] for _ in range(NCORES)] for w in range(NWIN)}
    segs_w = {w: [] for w in range(NWIN)}   # shared (cols, buid, first, last)
    ranks_of_dst = [np.full((NPC, 4), -1, np.int64) for _ in range(NCORES)]
    nrank_of_dst = [np.zeros(NPC, np.int64) for _ in range(NCORES)]
    dinvcols = [[] for _ in range(NCORES)]  # per block [128, ncols]

    for w in range(NWIN):
        for P in (0, 1):
            U = profiles[(w, P)]
            percore = secs[(w, P)]
            npairs = len(U)
            pos = 0
            while pos < npairs:
                take = min(BLKP, npairs - pos)
                blkp = _ceil(take, 128) * 128
                Ub = U[pos:pos + take]
                R = int(Ub[0]) if take else 0
                rankbase = used[P]
                buid = len(blocks)
                blocks.append(dict(P=P, rankbase=rankbase, blkp=blkp,
                                   ncols=blkp // 128))
                ncols_b = blkp // 128
                for c in range(NCORES):
                    ud, st, cnts, rs = percore[c]
                    nc_take = max(0, min(take, len(ud) - pos))
                    dsts = ud[pos:pos + nc_take]
                    nr = nrank_of_dst[c][dsts]
                    assert (nr < 4).all()
                    jj = np.arange(nc_take)
                    # ELL cell (p=j%128, cc=j//128) -> ptable row
                    # rankbase + p*ncols + cc (partition-major flush)
                    ranks_of_dst[c][dsts, nr] = (
                        P * PT_HALF + rankbase + (jj % 128) * ncols_b + jj // 128)
                    nrank_of_dst[c][dsts] = nr + 1
                    dcol = np.zeros(blkp, np.float32)
                    dcol[:nc_take] = dinv_g[c * NPC_REAL + dsts]
                    dinvcols[c].append(dcol.reshape(blkp // 128, 128).T)
                for r in range(R):
                    active = int(np.count_nonzero(Ub > r))
                    cols = _ceil(active, 128) if r > 0 else blkp // 128
                    segs_w[w].append((cols, buid, r == 0, r == R - 1))
                    for c in range(NCORES):
                        ud, st, cnts, rs = percore[c]
                        idx = np.full(cols * 128, ZERO_IDX, np.int64)
                        na = max(0, min(active, len(ud) - pos))
                        if na:
                            cb = cnts[pos:pos + na]
                            act = cb > r
                            idx[:na][act] = rs[st[pos:pos + na][act] + r]
                        streams[w][c].append(idx)
                used[P] += blkp
                pos += take
    assert used[0] + 128 <= PT_HALF and used[1] + 128 <= PT_HALF, used
    zero_rank = [used[0], used[1]]

    # slice window streams into calls (<= CALLN idx), shared structure
    acalls = []
    aidx = [[] for _ in range(NCORES)]
    for w in range(NWIN):
        rsegs = segs_w[w]
        if not rsegs:
            continue
        ri = 0
        roff = 0
        while ri < len(rsegs):
            n_cols = 0
            segs = []
            marks = []
            while ri < len(rsegs) and n_cols < CALLN // 128:
                cols, buid, first, last = rsegs[ri]
                avail = cols - roff
                takec = min(avail, CALLN // 128 - n_cols)
                segs.append((n_cols, takec, buid, roff, first,
                             last and (roff + takec == cols)))
                marks.append((ri, roff, takec))
                n_cols += takec
                roff += takec
                if roff == cols:
                    ri += 1
                    roff = 0
            acalls.append((w, n_cols * 128, segs))
            for c in range(NCORES):
                for (rj, ro, tk) in marks:
                    aidx[c].append(streams[w][c][rj][ro * 128:(ro + tk) * 128])
    aidx = [np.concatenate(a) for a in aidx]

    # phase B: per ptable half, slots in s-order, 4 ranks each (shared calls)
    bcalls = []
    bidx = [[] for _ in range(NCORES)]
    for P in (0, 1):
        slots = np.arange(NPC)
        slots = slots[((slots // 64) % 2) == P]
        idx_c = []
        for c in range(NCORES):
            ranks = ranks_of_dst[c][slots]
            zr = P * PT_HALF + zero_rank[P]
            ranks = np.where(ranks < 0, zr, ranks)
            ic = (ranks - P * PT_HALF).reshape(-1)
            assert ic.min() >= 0 and ic.max() < PT_HALF
            idx_c.append(ic)
        total = len(idx_c[0])
        off = 0
        while off < total:
            n = min(CALLN, total - off)
            chunks = []
            coff = 0
            while coff < n:
                cn = min(2048, n - coff)
                nq = cn // 4 // 64
                assert nq * 256 == cn, (cn, nq)
                a0 = (off + coff) // 4 // 64
                chunks.append((coff // 128, nq, a0))
                coff += cn
            bcalls.append((P, n, chunks))
            for c in range(NCORES):
                bidx[c].append(idx_c[c][off:off + n])
            off += n
    bidx = [np.concatenate(b) for b in bidx]

    plan = dict(acalls=acalls, bcalls=bcalls, blocks=blocks,
                zero_rank=zero_rank)
    cores = [dict(aidx=aidx[c], bidx=bidx[c], dinvcols=dinvcols[c])
             for c in range(NCORES)]

    EPC = E_OUT // NCORES
    fcalls, fpacked, fslotmap = _head_plan(edge_index_out, EPC)
    return dinv_g, plan, cores, fcalls, fpacked, fslotmap


def _head_plan(edge_index_out, EPC):
    fg = {}
    for c in range(NCORES):
        es = np.asarray(edge_index_out[0][c * EPC:(c + 1) * EPC], np.int64)
        ed = np.asarray(edge_index_out[1][c * EPC:(c + 1) * EPC], np.int64)
        allpos = np.concatenate([_row_of_global(es), _row_of_global(ed)])
        half = np.concatenate([np.zeros(EPC, np.int64), np.ones(EPC, np.int64)])
        eid = np.concatenate([np.arange(EPC), np.arange(EPC)])
        w = allpos // WINSZ
        key = half * NWIN + w
        order = np.lexsort((allpos, key))
        allpos, eid, half, key = allpos[order], eid[order], half[order], key[order]
        uk, starts = np.unique(key, return_index=True)
        starts = list(starts) + [len(allpos)]
        for i, k in enumerate(uk):
            fg.setdefault(int(k), [None] * NCORES)
            fg[int(k)][c] = (allpos[starts[i]:starts[i + 1]],
                             eid[starts[i]:starts[i + 1]],
                             half[starts[i]:starts[i + 1]])
    fcalls = []
    fpacked = [[] for _ in range(NCORES)]
    fslotmap = [[] for _ in range(NCORES)]
    CH = 2048
    for key in sorted(fg):
        w = key % NWIN
        base = w * WINSZ
        percore = fg[key]
        nmax = max((len(x[0]) if x is not None else 0) for x in percore)
        if nmax == 0:
            continue
        for ci in range(_ceil(nmax, CH)):
            size = min(CH, nmax - ci * CH)
            n_pad = _ceil(size, 16) * 16
            off = ci * CH
            for c in range(NCORES):
                ap_, eid_, half_ = percore[c] if percore[c] is not None else (
                    np.empty(0, np.int64), np.empty(0, np.int64),
                    np.empty(0, np.int64))
                spc = ap_[off:off + size]
                ei = eid_[off:off + size]
                hf = half_[off:off + size]
                npad = n_pad - len(spc)
                g = np.concatenate([spc - base, np.zeros(npad, np.int64)])
                assert g.min() >= 0 and g.max() < WINSZ
                fpacked[c].append(g)
                fslotmap[c].append((ei, hf))
            fcalls.append((w, n_pad))
    return fcalls, fpacked, fslotmap


def _build_program(plan, fcalls, repeat=1, body=True, no_phase_a=False,
                   no_phase_b=False):
    acalls, bcalls, blocks = plan["acalls"], plan["bcalls"], plan["blocks"]
    nc = bacc.Bacc("TRN2", target_bir_lowering=False, debug=False,
                   num_devices=NCORES, num_swdge_queues=4)
    GCA = sum(n // 16 for _, n, _ in acalls)
    GCB = sum(n // 16 for _, n, _ in bcalls)
    FC = sum(n // 16 for _, n in fcalls)
    TOTS = sum(_ceil(n, 128) * 128 for _, n in fcalls)
    NBLK = len(blocks)

    xT_in = nc.dram_tensor("xT", [IN_DIM, 2, HALFS], f32, kind="ExternalInput")
    dinvnm_in = nc.dram_tensor("dinvnm", [128, COLS], f32, kind="ExternalInput")
    bembT_in = nc.dram_tensor("bembT", [128, 1], f32, kind="ExternalInput")
    wemb_in = nc.dram_tensor("wemb", [IN_DIM, HID], f32, kind="ExternalInput")
    convw_in = nc.dram_tensor("convw", [128, L * HID], f32, kind="ExternalInput")
    bn_in = nc.dram_tensor("bn", [L, 128], f32, kind="ExternalInput")
    fcw_in = nc.dram_tensor("fcw", [128, 4], f32, kind="ExternalInput")
    dinvblk_in = nc.dram_tensor("dinvblk", [128, 16 * NBLK], f32,
                                kind="ExternalInput")
    gidxA_in = nc.dram_tensor("gidxA", [128, GCA], i16, kind="ExternalInput")
    gidxB_in = nc.dram_tensor("gidxB", [128, GCB], i16, kind="ExternalInput")
    fidx_in = nc.dram_tensor("fidx", [128, FC], i16, kind="ExternalInput")
    yout = nc.dram_tensor("yout", [TOTS, 4], f32, kind="ExternalOutput")

    with tile.TileContext(nc) as tc:
        with (
            tc.tile_pool(name="const", bufs=1) as cp,
            tc.tile_pool(name="xtp", bufs=1) as xtp,
            tc.tile_pool(name="big", bufs=1) as bp,
            tc.tile_pool(name="accp", bufs=2) as ap_,
            tc.tile_pool(name="msg", bufs=3) as mp,
            tc.tile_pool(name="work", bufs=2) as wp,
            tc.tile_pool(name="psA", bufs=2, space="PSUM") as psA,
            tc.tile_pool(name="psB", bufs=2, space="PSUM") as psB,
            tc.tile_pool(name="psT", bufs=1, space="PSUM") as psT,
            tc.tile_pool(name="dram", bufs=1, space="DRAM") as dp,
        ):
            # ---- DRAM internals
            bounce = dp.tile([NPC, HID], f32)
            table = dp.tile([TBLR, HID], f32)
            ptable = dp.tile([2 * PT_HALF, HID], f32)
            arb_in = dp.tile([2, HID], f32)
            arb_out = dp.tile([2, HID], f32)

            # ---- constants / inputs to SBUF
            id128 = cp.tile([128, 128], f32)
            make_identity(nc, id128[:])
            dinvnm = cp.tile([128, COLS], f32)
            nc.sync.dma_start(out=dinvnm[:], in_=dinvnm_in[:])
            bembT = cp.tile([128, 1], f32)
            nc.sync.dma_start(out=bembT[:], in_=bembT_in[:])
            wemb = cp.tile([IN_DIM, HID], f32)
            nc.sync.dma_start(out=wemb[:], in_=wemb_in[:])
            convw = cp.tile([128, L * HID], f32)
            nc.sync.dma_start(out=convw[:], in_=convw_in[:])
            fcw = cp.tile([128, 4], f32)
            nc.sync.dma_start(out=fcw[:], in_=fcw_in[:])
            dinvblk = cp.tile([128, 16 * NBLK], f32)
            nc.sync.dma_start(out=dinvblk[:], in_=dinvblk_in[:])
            gidxA = cp.tile([128, GCA], i16)
            nc.sync.dma_start(out=gidxA[:], in_=gidxA_in[:])
            gidxB = cp.tile([128, GCB], i16)
            nc.sync.dma_start(out=gidxB[:], in_=gidxB_in[:])
            S4 = cp.tile([128, 32], f32)
            # S4[p, g] = (p//4 == g): sum id128 columns in groups of 4
            id_v = id128[:].rearrange("p (g k) -> p g k", k=4)
            nc.vector.tensor_reduce(out=S4[:], in_=id_v,
                                    axis=mybir.AxisListType.X,
                                    op=mybir.AluOpType.add)

            _regs = {}

            def reg_of(n):
                if n not in _regs:
                    _regs[n] = nc.gpsimd.to_reg(n)
                return _regs[n]

            xeT = bp.tile([128, HALFS], f32)
            aggT = bp.tile([128, HALFS], f32)
            staging = bp.tile([128, COLS, HID], f32)
            aggT_v = aggT[:].rearrange("p (a b v) -> p a b v", b=2, v=32)

            # ---- pre-zero the ptable zero-rows (one per half), once
            ztile = wp.tile([128, 1, HID], f32, tag="ztile")
            nc.vector.memset(ztile[:], 0.0)
            for P in (0, 1):
                zr = P * PT_HALF + plan["zero_rank"][P]
                nc.sync.dma_start(
                    out=ptable[zr:zr + 128, :].rearrange("(p c) k -> p c k",
                                                         p=128),
                    in_=ztile[:, :, :])

            # ---- embed: xeT[64u+d, f] = sum_i x[i, slot] wemb[i, d] + bemb[d]
            EW = 256
            for u in (0, 1):
                xth = xtp.tile([IN_DIM, HALFS], f32, tag="xth")
                nc.sync.dma_start(out=xth[:], in_=xT_in[:, u, :])
                for fb in (range(0, HALFS, EW) if body else []):
                    wd = min(EW, HALFS - fb)
                    ps = psB.tile([128, EW], f32, tag="pb")
                    nc.tensor.matmul(out=ps[64 * u:64 * u + 64, :wd],
                                     lhsT=wemb[:], rhs=xth[:, fb:fb + wd],
                                     start=True, stop=True)
                    nc.vector.tensor_scalar_add(
                        out=xeT[64 * u:64 * u + 64, fb:fb + wd],
                        in0=ps[64 * u:64 * u + 64, :wd],
                        scalar1=bembT[64 * u:64 * u + 64, :])

            # ---- layers
            for l in ([li for _ in range(repeat) for li in range(L)]
                      if body else []):
                _emit_hw_blocks(nc, psA, xeT, convw, l * HID, staging, dinvnm)
                nc.sync.dma_start(
                    out=bounce[:].rearrange("(p c) k -> p c k", p=128),
                    in_=staging[:, :, :])
                nc.gpsimd.collective_compute(
                    "AllGather", mybir.AluOpType.bypass,
                    replica_groups=[list(range(NCORES))],
                    ins=[bounce[:].opt()], outs=[table[:].opt()])

                # ---- phase A
                if not no_phase_a:
                    off = 0
                    accs = {}
                    for k, (w, n, segs) in enumerate(acalls):
                        cc = n // 128
                        msg = mp.tile([128, CALLN // 128, HID], f32, tag="msg")
                        nc.gpsimd.dma_gather(
                            out_ap=msg[:, :cc, :],
                            in_ap=table[w * WINSZ:(w + 1) * WINSZ, :],
                            idxs_ap=gidxA[:, off:off + n // 16],
                            num_idxs=n, num_idxs_reg=reg_of(n), elem_size=HID,
                            single_packet=False, queue_num=k % 4)
                        off += n // 16
                        for (mcol, ncols, buid, acol, first, flush) in segs:
                            b = blocks[buid]
                            if first and acol == 0:
                                acc = ap_.tile([128, 16, HID], f32, tag="acc")
                                accs[buid] = acc
                            acc = accs[buid]
                            src_ap = msg[:, mcol:mcol + ncols, :]
                            dst_ap = acc[:, acol:acol + ncols, :]
                            if first:
                                nc.vector.tensor_copy(out=dst_ap, in_=src_ap)
                            else:
                                nc.vector.tensor_tensor(
                                    out=dst_ap, in0=dst_ap, in1=src_ap,
                                    op=mybir.AluOpType.add)
                            if flush:
                                nco = b["ncols"]
                                dv = dinvblk[:, 16 * buid:16 * buid + nco]
                                dv = dv.rearrange("p (c o) -> p c o", o=1)
                                nc.vector.tensor_tensor(
                                    out=acc[:, :nco, :], in0=acc[:, :nco, :],
                                    in1=dv.to_broadcast([128, nco, HID]),
                                    op=mybir.AluOpType.mult)
                                rb = b["P"] * PT_HALF + b["rankbase"]
                                nc.sync.dma_start(
                                    out=ptable[rb:rb + b["blkp"], :].rearrange(
                                        "(p c) k -> p c k", p=128),
                                    in_=acc[:, :nco, :])
                                del accs[buid]

                # ---- phase B
                if not no_phase_b:
                    off = 0
                    for k, (P, n, chunks) in enumerate(bcalls):
                        cc = n // 128
                        msg = mp.tile([128, CALLN // 128, HID], f32, tag="msg")
                        nc.gpsimd.dma_gather(
                            out_ap=msg[:, :cc, :],
                            in_ap=ptable[P * PT_HALF:(P + 1) * PT_HALF, :],
                            idxs_ap=gidxB[:, off:off + n // 16],
                            num_idxs=n, num_idxs_reg=reg_of(n), elem_size=HID,
                            single_packet=False, queue_num=k % 4)
                        off += n // 16
                        for (ccol, nq, a0) in chunks:
                            ps = psB.tile([128, 256], f32, tag="pb")
                            for i in range(nq):
                                lt = msg[:, ccol + 2 * i:ccol + 2 * i + 2, :]
                                nc.tensor.matmul(
                                    out=ps[:, 32 * i:32 * i + 32],
                                    lhsT=lt.rearrange("p a k -> p (a k)"),
                                    rhs=S4[:], start=True, stop=True)
                            nc.vector.tensor_copy(
                                out=aggT_v[:, a0:a0 + nq, P:P + 1, :],
                                in_=ps[:].rearrange("p (i o g) -> p i o g",
                                                    o=1, g=32)[:, :nq, :, :])

                # ---- BN stats (free-dim reduces on transposed agg)
                stats2 = wp.tile([128, 2], f32, tag="stats2")
                nc.vector.tensor_reduce(
                    out=stats2[:, 0:1],
                    in_=aggT[:].rearrange("p (o f) -> p o f", o=1),
                    axis=mybir.AxisListType.X, op=mybir.AluOpType.add)
                sqv = staging[:].rearrange("p c k -> p (c k)")
                nc.vector.tensor_tensor(out=sqv, in0=aggT[:], in1=aggT[:],
                                        op=mybir.AluOpType.mult)
                nc.vector.tensor_reduce(
                    out=stats2[:, 1:2],
                    in_=staging[:, :, :],
                    axis=mybir.AxisListType.XY, op=mybir.AluOpType.add)
                trp = psT.tile([128, 128], f32, tag="tr")
                psS = trp[0:2, :]
                nc.tensor.transpose(out=psS, in_=stats2[:], identity=id128[:])
                st2 = wp.tile([2, 128], f32, tag="st2")
                nc.vector.tensor_copy(out=st2[:], in_=psS)
                hc = wp.tile([2, HID], f32, tag="hc")
                nc.vector.tensor_tensor(out=hc[:], in0=st2[:, 0:HID],
                                        in1=st2[:, HID:128],
                                        op=mybir.AluOpType.add)
                nc.sync.dma_start(out=arb_in[:], in_=hc[:])
                nc.gpsimd.collective_compute(
                    "AllReduce", mybir.AluOpType.add,
                    replica_groups=[list(range(NCORES))],
                    ins=[arb_in[:].opt()], outs=[arb_out[:].opt()])
                gs = wp.tile([2, HID], f32, tag="gs")
                nc.sync.dma_start(out=gs[:], in_=arb_out[:])
                trq = psT.tile([128, 128], f32, tag="tr")
                psQ = trq[0:1, 0:HID]
                nc.tensor.matmul(out=psQ, lhsT=id128[0:2, 1:2], rhs=gs[:],
                                 start=True, stop=True)
                sq_row = wp.tile([1, HID], f32, tag="sqrow")
                nc.vector.tensor_scalar_mul(out=sq_row[:], in0=psQ,
                                            scalar1=1.0 / N)
                mean = wp.tile([1, HID], f32, tag="mean")
                nc.vector.tensor_scalar_mul(out=mean[:], in0=gs[0:1, :],
                                            scalar1=1.0 / N)
                var = wp.tile([1, HID], f32, tag="var")
                nc.vector.tensor_tensor(out=var[:], in0=mean[:], in1=mean[:],
                                        op=mybir.AluOpType.mult)
                nc.vector.tensor_tensor(out=var[:], in0=sq_row[:], in1=var[:],
                                        op=mybir.AluOpType.subtract)
                nc.vector.tensor_scalar_add(out=var[:], in0=var[:],
                                            scalar1=float(BN_EPS))
                sd = wp.tile([1, HID], f32, tag="sd")
                nc.scalar.activation(out=sd[:], in_=var[:],
                                     func=mybir.ActivationFunctionType.Sqrt)
                rs = wp.tile([1, HID], f32, tag="rs")
                nc.vector.reciprocal(out=rs[:], in_=sd[:])
                bnl = wp.tile([1, 128], f32, tag="bnl")
                nc.sync.dma_start(out=bnl[:], in_=bn_in[l:l + 1, :])
                srow = wp.tile([1, 128], f32, tag="srow")
                trow = wp.tile([1, 128], f32, tag="trow")
                nc.vector.tensor_tensor(out=srow[:, 0:HID], in0=bnl[:, 0:HID],
                                        in1=rs[:], op=mybir.AluOpType.mult)
                nc.vector.tensor_copy(out=srow[:, HID:128], in_=srow[:, 0:HID])
                tmp = wp.tile([1, HID], f32, tag="tmp")
                nc.vector.tensor_tensor(out=tmp[:], in0=mean[:],
                                        in1=srow[:, 0:HID],
                                        op=mybir.AluOpType.mult)
                nc.vector.tensor_tensor(out=trow[:, 0:HID], in0=bnl[:, HID:128],
                                        in1=tmp[:], op=mybir.AluOpType.subtract)
                nc.vector.tensor_copy(out=trow[:, HID:128], in_=trow[:, 0:HID])
                trc = psT.tile([128, 128], f32, tag="tr")
                nc.tensor.transpose(out=trc[:, 0:1], in_=srow[:],
                                    identity=id128[0:1, 0:1])
                nc.tensor.transpose(out=trc[:, 1:2], in_=trow[:],
                                    identity=id128[0:1, 0:1])
                stc = wp.tile([128, 2], f32, tag="stc")
                nc.vector.tensor_copy(out=stc[:], in_=trc[:, 0:2])

                # apply: xeT += relu(aggT * s + t)
                nc.vector.tensor_scalar(
                    out=aggT[:], in0=aggT[:], scalar1=stc[:, 0:1],
                    scalar2=stc[:, 1:2], op0=mybir.AluOpType.mult,
                    op1=mybir.AluOpType.add)
                nc.vector.tensor_scalar_max(out=aggT[:], in0=aggT[:],
                                            scalar1=0.0)
                nc.vector.tensor_tensor(out=xeT[:], in0=xeT[:], in1=aggT[:],
                                        op=mybir.AluOpType.add)

            # ---- head: y4 per node -> table -> AllGather -> edge gathers
            nc.vector.memset(staging[:], 0.0)
            for j in range(COLS if body else 0):
                ps = psA.tile([128, HID], f32, tag="hw")
                _emit_block_mm(nc, ps, xeT, fcw, 0, j, out_w=4)
                nc.vector.tensor_copy(out=staging[:, j, 0:4], in_=ps[:, 0:4])
            nc.sync.dma_start(
                out=bounce[:].rearrange("(p c) k -> p c k", p=128),
                in_=staging[:, :, :])
            nc.gpsimd.collective_compute(
                "AllGather", mybir.AluOpType.bypass,
                replica_groups=[list(range(NCORES))],
                ins=[bounce[:].opt()], outs=[table[:].opt()])

            assert FC <= GCA, (FC, GCA)
            nc.sync.dma_start(out=gidxA[:, :FC], in_=fidx_in[:])
            off = 0
            soff = 0
            for k, (w, n_pad) in enumerate(fcalls if body else []):
                cc = _ceil(n_pad, 128)
                msg = mp.tile([128, CALLN // 128, HID], f32, tag="msg")
                nc.gpsimd.dma_gather(
                    out_ap=msg[:, :cc, :],
                    in_ap=table[w * WINSZ:(w + 1) * WINSZ, :],
                    idxs_ap=gidxA[:, off:off + n_pad // 16],
                    num_idxs=n_pad, num_idxs_reg=reg_of(n_pad), elem_size=HID,
                    single_packet=False, queue_num=k % 4)
                yo = mp.tile([128, CALLN // 128, 4], f32, tag="yo")
                nc.vector.tensor_copy(out=yo[:, :cc, :], in_=msg[:, :cc, 0:4])
                nc.sync.dma_start(
                    out=yout[soff:soff + cc * 128, :].rearrange(
                        "(p c) k -> p c k", p=128),
                    in_=yo[:, :cc, :])
                off += n_pad // 16
                soff += cc * 128
            if not body:
                yo0 = mp.tile([128, CALLN // 128, 4], f32, tag="yo")
                nc.vector.memset(yo0[:], 0.0)
                nc.sync.dma_start(
                    out=yout[0:(CALLN // 128) * 128, :].rearrange(
                        "(p c) k -> p c k", p=128),
                    in_=yo0[:, :, :])
    nc.compile()
    return nc, TOTS


def _emit_hw_blocks(nc, psA, xeT, convw, cbase, staging, dinvnm):
    for j in range(COLS):
        ps = psA.tile([128, HID], f32, tag="hw")
        _emit_block_mm(nc, ps, xeT, convw, cbase, j, out_w=HID)
        nc.vector.tensor_scalar_mul(out=staging[:, j, :], in0=ps[:],
                                    scalar1=dinvnm[:, j:j + 1])


def _emit_block_mm(nc, ps, xeT, rhs_tile, cbase, j, out_w):
    """ps[p, :] = (xe @ W) for node block j; rhs_tile is [128, *] with both
    halves stacked (rows 0:64 and 64:128 hold the same weights)."""
    fb = 64 * j
    nc.tensor.matmul(out=ps[0:64, :out_w], lhsT=xeT[0:64, fb:fb + 64],
                     rhs=rhs_tile[0:64, cbase:cbase + out_w],
                     start=True, stop=True)
    nc.tensor.matmul(out=ps[64:128, :out_w], lhsT=xeT[64:128, fb:fb + 64],
                     rhs=rhs_tile[64:128, cbase:cbase + out_w],
                     start=True, stop=True)


def _pack_inputs(inputs, dinv_g, cores, plan, fpacked):
    x = np.asarray(inputs["x"], np.float32)
    W_emb = np.asarray(inputs["W_emb"], np.float32)
    b_emb = np.asarray(inputs["b_emb"], np.float32)
    conv_W = np.asarray(inputs["conv_W"], np.float32)
    bn_gamma = np.asarray(inputs["bn_gamma"], np.float32)
    bn_beta = np.asarray(inputs["bn_beta"], np.float32)
    fc_W = np.asarray(inputs["fc_W"], np.float32)

    convw = np.transpose(conv_W, (1, 0, 2)).reshape(HID, L * HID)
    convw2 = np.concatenate([convw, convw], axis=0)          # [128, L*64]
    fcw_cat = np.concatenate([fc_W[:HID], fc_W[HID:]], axis=1)  # [64, 4]
    fcw2 = np.concatenate([fcw_cat, fcw_cat], axis=0)        # [128, 4]
    bn_cat = np.concatenate([bn_gamma, bn_beta], axis=1)     # [L, 128]
    bembT = np.tile(b_emb, 2).reshape(128, 1)

    NBLK = len(plan["blocks"])
    in_maps = []
    for c in range(NCORES):
        core = cores[c]
        xs = np.zeros((NPC, IN_DIM), np.float32)
        xs[:NPC_REAL] = x[c * NPC_REAL:(c + 1) * NPC_REAL]
        dv = np.zeros(NPC, np.float32)
        dv[:NPC_REAL] = dinv_g[c * NPC_REAL:(c + 1) * NPC_REAL]
        # xT[i, u, f] = x[slot(u,f), i], slot = 64*(f//32) + 32*u + f%32
        f = np.arange(HALFS)
        slot_u0 = 64 * (f // 32) + (f % 32)
        slot_u1 = slot_u0 + 32
        xT = np.stack([xs[slot_u0].T, xs[slot_u1].T], axis=1)  # [16, 2, HALFS]
        dinvnm = dv[SLOT_OF_PJ]                                # [128, COLS]
        dinvblk = np.zeros((128, 16 * NBLK), np.float32)
        for buid, arr in enumerate(core["dinvcols"]):
            dinvblk[:, 16 * buid:16 * buid + arr.shape[1]] = arr
        in_maps.append(dict(
            xT=np.ascontiguousarray(xT),
            dinvnm=np.ascontiguousarray(dinvnm),
            bembT=bembT, wemb=W_emb, convw=np.ascontiguousarray(convw2),
            bn=bn_cat, fcw=fcw2,
            dinvblk=dinvblk,
            gidxA=_wrap16(core["aidx"]),
            gidxB=_wrap16(core["bidx"]),
            fidx=np.concatenate([_wrap16(a) for a in fpacked[c]], axis=1),
        ))
    return in_maps


def _prepare(inputs):
    edge_index = np.asarray(inputs["edge_index"])
    edge_index_out = np.asarray(inputs["edge_index_out"])
    key = hash((edge_index[0, :50].tobytes(), edge_index_out[0, :50].tobytes()))
    if key in _CACHE:
        return _CACHE[key]
    dinv_g, plan, cores, fcalls, fpacked, fslotmap = _preprocess(
        edge_index, edge_index_out)
    nc, TOTS = _build_program(plan, fcalls)
    _CACHE[key] = (dinv_g, plan, cores, fcalls, fpacked, fslotmap, nc, TOTS)
    return _CACHE[key]


def kernel(x, edge_index, edge_index_out, W_emb, b_emb, conv_W, conv_b,
           bn_gamma, bn_beta, fc_W, fc_b):
    (dinv_g, plan, cores, fcalls, fpacked, fslotmap, nc, TOTS) = _prepare(
        dict(edge_index=edge_index, edge_index_out=edge_index_out))
    in_maps = _pack_inputs(
        dict(x=x, W_emb=W_emb, b_emb=b_emb, conv_W=conv_W, bn_gamma=bn_gamma,
             bn_beta=bn_beta, fc_W=fc_W),
        dinv_g, cores, plan, fpacked)
    res = run_bass_kernel_spmd(nc, in_maps, core_ids=list(range(NCORES)))

    EPC = E_OUT // NCORES
    out = np.zeros((E_OUT, OUT_DIM), np.float32)
    fc_b = np.asarray(fc_b, np.float32)
    for c in range(NCORES):
        y = res.results[c]["yout"]
        soff = 0
        for k, (w, n_pad) in enumerate(fcalls):
            cc = _ceil(n_pad, 128)
            eid, half = fslotmap[c][k]
            nreal = len(eid)
            i = np.arange(nreal)
            rows = soff + (i % 128) * cc + i // 128
            vals = y[rows]
            sel_src = half == 0
            out[c * EPC + eid[sel_src], :] += vals[sel_src][:, 0:2]
            out[c * EPC + eid[~sel_src], :] += vals[~sel_src][:, 2:4]
            soff += cc * 128
    out += fc_b[None, :]
    return out


# revision 36
# speedup vs baseline: 1.4567x; 1.4391x over previous
"""8-core TRN2 Bass kernel for the 6-layer GCN edge classifier — gather-only.

Per layer (nodes dst-sharded 8 ways, 12928 slots/core incl 428 phantoms):
- hw' = dinv * (xe @ W_l) computed from transposed xeT via per-block matmuls,
  staged node-major, AllGather -> replicated table [103424, 64] in HBM.
- Phase A: ELL-style gather. (dst, src-window) pairs sorted by degree, blocks
  of <=2048 pairs, round-r gathers the r-th edge of every still-active pair
  (prefix of the block). Rounds accumulate on DVE into an SBUF acc tile;
  at block end acc *= dinv[dst] and is flushed to ptable (HBM, 2x 32768-row
  halves so combine indices fit int16).
- Phase B: every dst has exactly 4 ptable partials (zero-padded). One gather
  pass + 2-column matmuls against a fixed S4 matrix reduce them, emitting
  aggT [128, 6464] transposed (partition = 64*u + dim, interleaved halves).
- BN stats via free-dim reduce + tiny AllReduce; fused affine + relu +
  residual on DVE, all 128 partitions wide.
- Head: y4 = [xe@fcW_top | xe@fcW_bot] per node -> table -> AllGather ->
  per-edge gathers; host adds halves + bias.

No dma_scatter_add anywhere (it dominated the previous version's runtime).
"""
import sys
import os

for _p in ("/opt/trn_rl_repo", "/root/.axon_site/_ro/trn_rl_repo"):
    if os.path.isdir(_p) and _p not in sys.path:
        sys.path.insert(0, _p)

import numpy as np
import concourse.bass as bass
import concourse.mybir as mybir
import concourse.tile as tile
from concourse import bacc
from concourse.bass_utils import run_bass_kernel_spmd
from concourse.masks import make_identity

# problem constants
N = 100000
E = 1600000
E_OUT = 400000
IN_DIM = 16
HID = 64
OUT_DIM = 2
L = 6
BN_EPS = 1e-5

NCORES = 8
NPC_REAL = 12500
NPC = 12928                # slots per core (428 phantoms)
COLS = NPC // 128          # 101 node blocks
TBLR = NCORES * NPC        # 103424 table rows
NWIN = 4
WINSZ = TBLR // NWIN       # 25856 (< 32768, int16-safe)
PT_HALF = 32768            # ptable half (window) size
CALLN = 4096               # max idx per gather call
BLKP = 2048                # max pairs per ELL block
HALFS = NPC // 2           # 6464 slots per partition-half

f32 = mybir.dt.float32
i16 = mybir.dt.int16

# slot <-> table-row permutation (from the hw'-block matmul layout):
# staging col j partition p holds slot S = 128j + 64*((p%64)//32) + 32*(p//64)
# + (p%32); bounce/table row r = p*COLS + j (partition-major so each SBUF
# partition's DMA stream is one contiguous block -> few fat descriptors).
_P = np.arange(128)
_J = np.arange(COLS)
SLOT_OF_PJ = (128 * _J[None, :] + 64 * ((_P[:, None] % 64) // 32)
              + 32 * (_P[:, None] // 64) + (_P[:, None] % 32))   # [128, COLS]
ROW_OF_SLOT = np.empty(NPC, np.int64)
ROW_OF_SLOT[SLOT_OF_PJ.reshape(-1)] = (COLS * np.repeat(np.arange(128), COLS)
                                       + np.tile(np.arange(COLS), 128))
assert np.array_equal(np.sort(SLOT_OF_PJ.reshape(-1)), np.arange(NPC))
ZERO_IDX = int(ROW_OF_SLOT[NPC_REAL])   # a phantom slot's row (< NPC < WINSZ)
# all-zero table rows usable for padding, valid in every window (both cores
# of a window contribute their 428 phantom slots). Cycling over them avoids
# hammering a single HBM row with padding gathers (measured 2x+ slowdown).
ZPOOL = np.concatenate([ROW_OF_SLOT[NPC_REAL:], ROW_OF_SLOT[NPC_REAL:] + NPC])
assert ZPOOL.max() < WINSZ

_CACHE = {}


def _ceil(a, b):
    return -(-a // b)


def _wrap16(idx):
    """[n] int array (n % 16 == 0) -> [128, n//16] int16 for Ant DMA ops."""
    n = idx.shape[0]
    w = idx.reshape(n // 16, 16).T.astype(np.int16)
    return np.tile(w, (8, 1))


def _row_of_global(v):
    """global node id -> table row."""
    v = np.asarray(v, np.int64)
    return (v // NPC_REAL) * NPC + ROW_OF_SLOT[v % NPC_REAL]


def _preprocess(edge_index, edge_index_out):
    src = np.asarray(edge_index[0], dtype=np.int64)
    dst = np.asarray(edge_index[1], dtype=np.int64)
    deg = np.bincount(dst, minlength=N).astype(np.float64) + 1.0
    dinv_g = (1.0 / np.sqrt(deg)).astype(np.float32)

    # pass 1: per core, per section (w, P): pairs (dst, srcs) sorted by count
    # desc. secs[(w, P)][c] = (dsts, starts, counts_sorted_desc, rs)
    secs = {(w, P): [None] * NCORES for w in range(NWIN) for P in (0, 1)}
    for c in range(NCORES):
        m = (dst // NPC_REAL) == c
        s_g = np.concatenate([src[m], c * NPC_REAL + np.arange(NPC_REAL)])
        d_loc = np.concatenate([dst[m] - c * NPC_REAL, np.arange(NPC_REAL)])
        rows = _row_of_global(s_g)
        w_e = rows // WINSZ
        rb_e = rows - w_e * WINSZ
        p_half = (d_loc // 64) % 2
        order = np.lexsort((d_loc, p_half, w_e))
        w_e, rb_e, d2, p2 = w_e[order], rb_e[order], d_loc[order], p_half[order]
        for w in range(NWIN):
            for P in (0, 1):
                sel = (w_e == w) & (p2 == P)
                ds, rs = d2[sel], rb_e[sel]
                ud, starts, counts = np.unique(ds, return_index=True,
                                               return_counts=True)
                po = np.argsort(-counts, kind="stable")
                secs[(w, P)][c] = (ud[po], starts[po], counts[po], rs)

    # unified degree profile per section: U_j = max over cores of counts[j]
    profiles = {}
    for key, percore in secs.items():
        U_len = max(len(x[0]) for x in percore)
        U = np.zeros(U_len, np.int64)
        for (_, _, counts, _) in percore:
            U[:len(counts)] = np.maximum(U[:len(counts)], counts)
        profiles[key] = U

    # pass 2: shared block structure + per-core idx streams
    used = [0, 0]
    blocks = []                     # shared: P, rankbase, blkp, ncols
    streams = {w: [[] for _ in range(NCORES)] for w in range(NWIN)}
    segs_w = {w: [] for w in range(NWIN)}   # shared (cols, buid, first, last)
    ranks_of_dst = [np.full((NPC, 4), -1, np.int64) for _ in range(NCORES)]
    nrank_of_dst = [np.zeros(NPC, np.int64) for _ in range(NCORES)]
    dinvcols = [[] for _ in range(NCORES)]  # per block [128, ncols]

    for w in range(NWIN):
        for P in (0, 1):
            U = profiles[(w, P)]
            percore = secs[(w, P)]
            npairs = len(U)
            pos = 0
            while pos < npairs:
                take = min(BLKP, npairs - pos)
                blkp = _ceil(take, 128) * 128
                Ub = U[pos:pos + take]
                R = int(Ub[0]) if take else 0
                rankbase = used[P]
                buid = len(blocks)
                blocks.append(dict(P=P, rankbase=rankbase, blkp=blkp,
                                   ncols=blkp // 128))
                ncols_b = blkp // 128
                for c in range(NCORES):
                    ud, st, cnts, rs = percore[c]
                    nc_take = max(0, min(take, len(ud) - pos))
                    dsts = ud[pos:pos + nc_take]
                    nr = nrank_of_dst[c][dsts]
                    assert (nr < 4).all()
                    jj = np.arange(nc_take)
                    # ELL cell (p=j%128, cc=j//128) -> ptable row
                    # rankbase + p*ncols + cc (partition-major flush)
                    ranks_of_dst[c][dsts, nr] = (
                        P * PT_HALF + rankbase + (jj % 128) * ncols_b + jj // 128)
                    nrank_of_dst[c][dsts] = nr + 1
                    dcol = np.zeros(blkp, np.float32)
                    dcol[:nc_take] = dinv_g[c * NPC_REAL + dsts]
                    dinvcols[c].append(dcol.reshape(blkp // 128, 128).T)
                for r in range(R):
                    active = int(np.count_nonzero(Ub > r))
                    cols = _ceil(active, 128) if r > 0 else blkp // 128
                    segs_w[w].append((cols, buid, r == 0, r == R - 1))
                    for c in range(NCORES):
                        ud, st, cnts, rs = percore[c]
                        idx = ZPOOL[(np.arange(cols * 128) + r * 131)
                                    % len(ZPOOL)].copy()
                        na = max(0, min(active, len(ud) - pos))
                        if na:
                            cb = cnts[pos:pos + na]
                            act = cb > r
                            idx[:na][act] = rs[st[pos:pos + na][act] + r]
                        streams[w][c].append(idx)
                used[P] += blkp
                pos += take
    assert used[0] + 128 <= PT_HALF and used[1] + 128 <= PT_HALF, used
    zero_rank = [used[0], used[1]]

    # slice window streams into calls (<= CALLN idx), shared structure
    acalls = []
    aidx = [[] for _ in range(NCORES)]
    for w in range(NWIN):
        rsegs = segs_w[w]
        if not rsegs:
            continue
        ri = 0
        roff = 0
        while ri < len(rsegs):
            n_cols = 0
            segs = []
            marks = []
            while ri < len(rsegs) and n_cols < CALLN // 128:
                cols, buid, first, last = rsegs[ri]
                avail = cols - roff
                takec = min(avail, CALLN // 128 - n_cols)
                segs.append((n_cols, takec, buid, roff, first,
                             last and (roff + takec == cols)))
                marks.append((ri, roff, takec))
                n_cols += takec
                roff += takec
                if roff == cols:
                    ri += 1
                    roff = 0
            acalls.append((w, n_cols * 128, segs))
            for c in range(NCORES):
                for (rj, ro, tk) in marks:
                    aidx[c].append(streams[w][c][rj][ro * 128:(ro + tk) * 128])
    aidx = [np.concatenate(a) for a in aidx]

    # phase B: per ptable half, slots in s-order, 4 ranks each (shared calls)
    bcalls = []
    bidx = [[] for _ in range(NCORES)]
    for P in (0, 1):
        slots = np.arange(NPC)
        slots = slots[((slots // 64) % 2) == P]
        idx_c = []
        for c in range(NCORES):
            ranks = ranks_of_dst[c][slots]
            zr = P * PT_HALF + zero_rank[P]
            zcyc = zr + (np.arange(ranks.size).reshape(ranks.shape) % 128)
            ranks = np.where(ranks < 0, zcyc, ranks)
            ic = (ranks - P * PT_HALF).reshape(-1)
            assert ic.min() >= 0 and ic.max() < PT_HALF
            idx_c.append(ic)
        total = len(idx_c[0])
        off = 0
        while off < total:
            n = min(CALLN, total - off)
            chunks = []
            coff = 0
            while coff < n:
                cn = min(2048, n - coff)
                nq = cn // 4 // 64
                assert nq * 256 == cn, (cn, nq)
                a0 = (off + coff) // 4 // 64
                chunks.append((coff // 128, nq, a0))
                coff += cn
            bcalls.append((P, n, chunks))
            for c in range(NCORES):
                bidx[c].append(idx_c[c][off:off + n])
            off += n
    bidx = [np.concatenate(b) for b in bidx]

    plan = dict(acalls=acalls, bcalls=bcalls, blocks=blocks,
                zero_rank=zero_rank)
    cores = [dict(aidx=aidx[c], bidx=bidx[c], dinvcols=dinvcols[c])
             for c in range(NCORES)]

    EPC = E_OUT // NCORES
    fcalls, fpacked, fslotmap = _head_plan(edge_index_out, EPC)
    return dinv_g, plan, cores, fcalls, fpacked, fslotmap


def _head_plan(edge_index_out, EPC):
    fg = {}
    for c in range(NCORES):
        es = np.asarray(edge_index_out[0][c * EPC:(c + 1) * EPC], np.int64)
        ed = np.asarray(edge_index_out[1][c * EPC:(c + 1) * EPC], np.int64)
        allpos = np.concatenate([_row_of_global(es), _row_of_global(ed)])
        half = np.concatenate([np.zeros(EPC, np.int64), np.ones(EPC, np.int64)])
        eid = np.concatenate([np.arange(EPC), np.arange(EPC)])
        w = allpos // WINSZ
        key = half * NWIN + w
        order = np.lexsort((allpos, key))
        allpos, eid, half, key = allpos[order], eid[order], half[order], key[order]
        uk, starts = np.unique(key, return_index=True)
        starts = list(starts) + [len(allpos)]
        for i, k in enumerate(uk):
            fg.setdefault(int(k), [None] * NCORES)
            fg[int(k)][c] = (allpos[starts[i]:starts[i + 1]],
                             eid[starts[i]:starts[i + 1]],
                             half[starts[i]:starts[i + 1]])
    fcalls = []
    fpacked = [[] for _ in range(NCORES)]
    fslotmap = [[] for _ in range(NCORES)]
    CH = 2048
    for key in sorted(fg):
        w = key % NWIN
        base = w * WINSZ
        percore = fg[key]
        nmax = max((len(x[0]) if x is not None else 0) for x in percore)
        if nmax == 0:
            continue
        for ci in range(_ceil(nmax, CH)):
            size = min(CH, nmax - ci * CH)
            n_pad = _ceil(size, 16) * 16
            off = ci * CH
            for c in range(NCORES):
                ap_, eid_, half_ = percore[c] if percore[c] is not None else (
                    np.empty(0, np.int64), np.empty(0, np.int64),
                    np.empty(0, np.int64))
                spc = ap_[off:off + size]
                ei = eid_[off:off + size]
                hf = half_[off:off + size]
                npad = n_pad - len(spc)
                g = np.concatenate([spc - base,
                                    ZPOOL[np.arange(npad) % len(ZPOOL)]])
                assert g.min() >= 0 and g.max() < WINSZ
                fpacked[c].append(g)
                fslotmap[c].append((ei, hf))
            fcalls.append((w, n_pad))
    return fcalls, fpacked, fslotmap


def _build_program(plan, fcalls, repeat=1, body=True, no_phase_a=False,
                   no_phase_b=False, a_adds=True, a_flush=True):
    acalls, bcalls, blocks = plan["acalls"], plan["bcalls"], plan["blocks"]
    nc = bacc.Bacc("TRN2", target_bir_lowering=False, debug=False,
                   num_devices=NCORES, num_swdge_queues=4)
    GCA = sum(n // 16 for _, n, _ in acalls)
    GCB = sum(n // 16 for _, n, _ in bcalls)
    FC = sum(n // 16 for _, n in fcalls)
    TOTS = sum(_ceil(n, 128) * 128 for _, n in fcalls)
    NBLK = len(blocks)

    xT_in = nc.dram_tensor("xT", [IN_DIM, 2, HALFS], f32, kind="ExternalInput")
    dinvnm_in = nc.dram_tensor("dinvnm", [128, COLS], f32, kind="ExternalInput")
    bembT_in = nc.dram_tensor("bembT", [128, 1], f32, kind="ExternalInput")
    wemb_in = nc.dram_tensor("wemb", [IN_DIM, HID], f32, kind="ExternalInput")
    convw_in = nc.dram_tensor("convw", [128, L * HID], f32, kind="ExternalInput")
    bn_in = nc.dram_tensor("bn", [L, 128], f32, kind="ExternalInput")
    fcw_in = nc.dram_tensor("fcw", [128, 4], f32, kind="ExternalInput")
    dinvblk_in = nc.dram_tensor("dinvblk", [128, 16 * NBLK], f32,
                                kind="ExternalInput")
    gidxA_in = nc.dram_tensor("gidxA", [128, GCA], i16, kind="ExternalInput")
    gidxB_in = nc.dram_tensor("gidxB", [128, GCB], i16, kind="ExternalInput")
    fidx_in = nc.dram_tensor("fidx", [128, FC], i16, kind="ExternalInput")
    yout = nc.dram_tensor("yout", [TOTS, 4], f32, kind="ExternalOutput")

    with tile.TileContext(nc) as tc:
        with (
            tc.tile_pool(name="const", bufs=1) as cp,
            tc.tile_pool(name="xtp", bufs=1) as xtp,
            tc.tile_pool(name="big", bufs=1) as bp,
            tc.tile_pool(name="accp", bufs=2) as ap_,
            tc.tile_pool(name="msg", bufs=3) as mp,
            tc.tile_pool(name="work", bufs=2) as wp,
            tc.tile_pool(name="psA", bufs=2, space="PSUM") as psA,
            tc.tile_pool(name="psB", bufs=2, space="PSUM") as psB,
            tc.tile_pool(name="psT", bufs=1, space="PSUM") as psT,
            tc.tile_pool(name="dram", bufs=1, space="DRAM") as dp,
        ):
            # ---- DRAM internals
            bounce = dp.tile([NPC, HID], f32)
            table = dp.tile([TBLR, HID], f32)
            ptable = dp.tile([2 * PT_HALF, HID], f32)
            arb_in = dp.tile([2, HID], f32)
            arb_out = dp.tile([2, HID], f32)

            # ---- constants / inputs to SBUF
            id128 = cp.tile([128, 128], f32)
            make_identity(nc, id128[:])
            dinvnm = cp.tile([128, COLS], f32)
            nc.sync.dma_start(out=dinvnm[:], in_=dinvnm_in[:])
            bembT = cp.tile([128, 1], f32)
            nc.sync.dma_start(out=bembT[:], in_=bembT_in[:])
            wemb = cp.tile([IN_DIM, HID], f32)
            nc.sync.dma_start(out=wemb[:], in_=wemb_in[:])
            convw = cp.tile([128, L * HID], f32)
            nc.sync.dma_start(out=convw[:], in_=convw_in[:])
            fcw = cp.tile([128, 4], f32)
            nc.sync.dma_start(out=fcw[:], in_=fcw_in[:])
            dinvblk = cp.tile([128, 16 * NBLK], f32)
            nc.sync.dma_start(out=dinvblk[:], in_=dinvblk_in[:])
            gidxA = cp.tile([128, GCA], i16)
            nc.sync.dma_start(out=gidxA[:], in_=gidxA_in[:])
            gidxB = cp.tile([128, GCB], i16)
            nc.sync.dma_start(out=gidxB[:], in_=gidxB_in[:])
            S4 = cp.tile([128, 32], f32)
            # S4[p, g] = (p//4 == g): sum id128 columns in groups of 4
            id_v = id128[:].rearrange("p (g k) -> p g k", k=4)
            nc.vector.tensor_reduce(out=S4[:], in_=id_v,
                                    axis=mybir.AxisListType.X,
                                    op=mybir.AluOpType.add)

            _regs = {}

            def reg_of(n):
                if n not in _regs:
                    _regs[n] = nc.gpsimd.to_reg(n)
                return _regs[n]

            xeT = bp.tile([128, HALFS], f32)
            aggT = bp.tile([128, HALFS], f32)
            staging = bp.tile([128, COLS, HID], f32)
            aggT_v = aggT[:].rearrange("p (a b v) -> p a b v", b=2, v=32)

            # ---- pre-zero the ptable zero-rows (one per half), once
            ztile = wp.tile([128, 1, HID], f32, tag="ztile")
            nc.vector.memset(ztile[:], 0.0)
            for P in (0, 1):
                zr = P * PT_HALF + plan["zero_rank"][P]
                nc.sync.dma_start(
                    out=ptable[zr:zr + 128, :].rearrange("(p c) k -> p c k",
                                                         p=128),
                    in_=ztile[:, :, :])

            # ---- embed: xeT[64u+d, f] = sum_i x[i, slot] wemb[i, d] + bemb[d]
            EW = 256
            for u in (0, 1):
                xth = xtp.tile([IN_DIM, HALFS], f32, tag="xth")
                nc.sync.dma_start(out=xth[:], in_=xT_in[:, u, :])
                for fb in (range(0, HALFS, EW) if body else []):
                    wd = min(EW, HALFS - fb)
                    ps = psB.tile([128, EW], f32, tag="pb")
                    nc.tensor.matmul(out=ps[64 * u:64 * u + 64, :wd],
                                     lhsT=wemb[:], rhs=xth[:, fb:fb + wd],
                                     start=True, stop=True)
                    nc.vector.tensor_scalar_add(
                        out=xeT[64 * u:64 * u + 64, fb:fb + wd],
                        in0=ps[64 * u:64 * u + 64, :wd],
                        scalar1=bembT[64 * u:64 * u + 64, :])

            # ---- layers
            for l in ([li for _ in range(repeat) for li in range(L)]
                      if body else []):
                _emit_hw_blocks(nc, psA, xeT, convw, l * HID, staging, dinvnm)
                nc.sync.dma_start(
                    out=bounce[:].rearrange("(p c) k -> p c k", p=128),
                    in_=staging[:, :, :])
                nc.gpsimd.collective_compute(
                    "AllGather", mybir.AluOpType.bypass,
                    replica_groups=[list(range(NCORES))],
                    ins=[bounce[:].opt()], outs=[table[:].opt()])

                # ---- phase A
                if not no_phase_a:
                    off = 0
                    accs = {}
                    for k, (w, n, segs) in enumerate(acalls):
                        cc = n // 128
                        msg = mp.tile([128, CALLN // 128, HID], f32, tag="msg")
                        nc.gpsimd.dma_gather(
                            out_ap=msg[:, :cc, :],
                            in_ap=table[w * WINSZ:(w + 1) * WINSZ, :],
                            idxs_ap=gidxA[:, off:off + n // 16],
                            num_idxs=n, num_idxs_reg=reg_of(n), elem_size=HID,
                            single_packet=False, queue_num=k % 4)
                        off += n // 16
                        if not a_adds:
                            continue
                        for (mcol, ncols, buid, acol, first, flush) in segs:
                            b = blocks[buid]
                            if first and acol == 0:
                                acc = ap_.tile([128, 16, HID], f32, tag="acc")
                                accs[buid] = acc
                            acc = accs[buid]
                            src_ap = msg[:, mcol:mcol + ncols, :]
                            dst_ap = acc[:, acol:acol + ncols, :]
                            if first:
                                nc.vector.tensor_copy(out=dst_ap, in_=src_ap)
                            else:
                                nc.vector.tensor_tensor(
                                    out=dst_ap, in0=dst_ap, in1=src_ap,
                                    op=mybir.AluOpType.add)
                            if flush and a_flush:
                                nco = b["ncols"]
                                dv = dinvblk[:, 16 * buid:16 * buid + nco]
                                dv = dv.rearrange("p (c o) -> p c o", o=1)
                                nc.vector.tensor_tensor(
                                    out=acc[:, :nco, :], in0=acc[:, :nco, :],
                                    in1=dv.to_broadcast([128, nco, HID]),
                                    op=mybir.AluOpType.mult)
                                rb = b["P"] * PT_HALF + b["rankbase"]
                                nc.sync.dma_start(
                                    out=ptable[rb:rb + b["blkp"], :].rearrange(
                                        "(p c) k -> p c k", p=128),
                                    in_=acc[:, :nco, :])
                                del accs[buid]

                # ---- phase B
                if not no_phase_b:
                    off = 0
                    for k, (P, n, chunks) in enumerate(bcalls):
                        cc = n // 128
                        msg = mp.tile([128, CALLN // 128, HID], f32, tag="msg")
                        nc.gpsimd.dma_gather(
                            out_ap=msg[:, :cc, :],
                            in_ap=ptable[P * PT_HALF:(P + 1) * PT_HALF, :],
                            idxs_ap=gidxB[:, off:off + n // 16],
                            num_idxs=n, num_idxs_reg=reg_of(n), elem_size=HID,
                            single_packet=False, queue_num=k % 4)
                        off += n // 16
                        for (ccol, nq, a0) in chunks:
                            ps = psB.tile([128, 256], f32, tag="pb")
                            for i in range(nq):
                                lt = msg[:, ccol + 2 * i:ccol + 2 * i + 2, :]
                                nc.tensor.matmul(
                                    out=ps[:, 32 * i:32 * i + 32],
                                    lhsT=lt.rearrange("p a k -> p (a k)"),
                                    rhs=S4[:], start=True, stop=True)
                            nc.vector.tensor_copy(
                                out=aggT_v[:, a0:a0 + nq, P:P + 1, :],
                                in_=ps[:].rearrange("p (i o g) -> p i o g",
                                                    o=1, g=32)[:, :nq, :, :])

                # ---- BN stats (free-dim reduces on transposed agg)
                stats2 = wp.tile([128, 2], f32, tag="stats2")
                nc.vector.tensor_reduce(
                    out=stats2[:, 0:1],
                    in_=aggT[:].rearrange("p (o f) -> p o f", o=1),
                    axis=mybir.AxisListType.X, op=mybir.AluOpType.add)
                sqv = staging[:].rearrange("p c k -> p (c k)")
                nc.vector.tensor_tensor(out=sqv, in0=aggT[:], in1=aggT[:],
                                        op=mybir.AluOpType.mult)
                nc.vector.tensor_reduce(
                    out=stats2[:, 1:2],
                    in_=staging[:, :, :],
                    axis=mybir.AxisListType.XY, op=mybir.AluOpType.add)
                trp = psT.tile([128, 128], f32, tag="tr")
                psS = trp[0:2, :]
                nc.tensor.transpose(out=psS, in_=stats2[:], identity=id128[:])
                st2 = wp.tile([2, 128], f32, tag="st2")
                nc.vector.tensor_copy(out=st2[:], in_=psS)
                hc = wp.tile([2, HID], f32, tag="hc")
                nc.vector.tensor_tensor(out=hc[:], in0=st2[:, 0:HID],
                                        in1=st2[:, HID:128],
                                        op=mybir.AluOpType.add)
                nc.sync.dma_start(out=arb_in[:], in_=hc[:])
                nc.gpsimd.collective_compute(
                    "AllReduce", mybir.AluOpType.add,
                    replica_groups=[list(range(NCORES))],
                    ins=[arb_in[:].opt()], outs=[arb_out[:].opt()])
                gs = wp.tile([2, HID], f32, tag="gs")
                nc.sync.dma_start(out=gs[:], in_=arb_out[:])
                trq = psT.tile([128, 128], f32, tag="tr")
                psQ = trq[0:1, 0:HID]
                nc.tensor.matmul(out=psQ, lhsT=id128[0:2, 1:2], rhs=gs[:],
                                 start=True, stop=True)
                sq_row = wp.tile([1, HID], f32, tag="sqrow")
                nc.vector.tensor_scalar_mul(out=sq_row[:], in0=psQ,
                                            scalar1=1.0 / N)
                mean = wp.tile([1, HID], f32, tag="mean")
                nc.vector.tensor_scalar_mul(out=mean[:], in0=gs[0:1, :],
                                            scalar1=1.0 / N)
                var = wp.tile([1, HID], f32, tag="var")
                nc.vector.tensor_tensor(out=var[:], in0=mean[:], in1=mean[:],
                                        op=mybir.AluOpType.mult)
                nc.vector.tensor_tensor(out=var[:], in0=sq_row[:], in1=var[:],
                                        op=mybir.AluOpType.subtract)
                nc.vector.tensor_scalar_add(out=var[:], in0=var[:],
                                            scalar1=float(BN_EPS))
                sd = wp.tile([1, HID], f32, tag="sd")
                nc.scalar.activation(out=sd[:], in_=var[:],
                                     func=mybir.ActivationFunctionType.Sqrt)
                rs = wp.tile([1, HID], f32, tag="rs")
                nc.vector.reciprocal(out=rs[:], in_=sd[:])
                bnl = wp.tile([1, 128], f32, tag="bnl")
                nc.sync.dma_start(out=bnl[:], in_=bn_in[l:l + 1, :])
                srow = wp.tile([1, 128], f32, tag="srow")
                trow = wp.tile([1, 128], f32, tag="trow")
                nc.vector.tensor_tensor(out=srow[:, 0:HID], in0=bnl[:, 0:HID],
                                        in1=rs[:], op=mybir.AluOpType.mult)
                nc.vector.tensor_copy(out=srow[:, HID:128], in_=srow[:, 0:HID])
                tmp = wp.tile([1, HID], f32, tag="tmp")
                nc.vector.tensor_tensor(out=tmp[:], in0=mean[:],
                                        in1=srow[:, 0:HID],
                                        op=mybir.AluOpType.mult)
                nc.vector.tensor_tensor(out=trow[:, 0:HID], in0=bnl[:, HID:128],
                                        in1=tmp[:], op=mybir.AluOpType.subtract)
                nc.vector.tensor_copy(out=trow[:, HID:128], in_=trow[:, 0:HID])
                trc = psT.tile([128, 128], f32, tag="tr")
                nc.tensor.transpose(out=trc[:, 0:1], in_=srow[:],
                                    identity=id128[0:1, 0:1])
                nc.tensor.transpose(out=trc[:, 1:2], in_=trow[:],
                                    identity=id128[0:1, 0:1])
                stc = wp.tile([128, 2], f32, tag="stc")
                nc.vector.tensor_copy(out=stc[:], in_=trc[:, 0:2])

                # apply: xeT += relu(aggT * s + t)
                nc.vector.tensor_scalar(
                    out=aggT[:], in0=aggT[:], scalar1=stc[:, 0:1],
                    scalar2=stc[:, 1:2], op0=mybir.AluOpType.mult,
                    op1=mybir.AluOpType.add)
                nc.vector.tensor_scalar_max(out=aggT[:], in0=aggT[:],
                                            scalar1=0.0)
                nc.vector.tensor_tensor(out=xeT[:], in0=xeT[:], in1=aggT[:],
                                        op=mybir.AluOpType.add)

            # ---- head: y4 per node -> table -> AllGather -> edge gathers
            nc.vector.memset(staging[:], 0.0)
            for j in range(COLS if body else 0):
                ps = psA.tile([128, HID], f32, tag="hw")
                _emit_block_mm(nc, ps, xeT, fcw, 0, j, out_w=4)
                nc.vector.tensor_copy(out=staging[:, j, 0:4], in_=ps[:, 0:4])
            nc.sync.dma_start(
                out=bounce[:].rearrange("(p c) k -> p c k", p=128),
                in_=staging[:, :, :])
            nc.gpsimd.collective_compute(
                "AllGather", mybir.AluOpType.bypass,
                replica_groups=[list(range(NCORES))],
                ins=[bounce[:].opt()], outs=[table[:].opt()])

            assert FC <= GCA, (FC, GCA)
            nc.sync.dma_start(out=gidxA[:, :FC], in_=fidx_in[:])
            off = 0
            soff = 0
            for k, (w, n_pad) in enumerate(fcalls if body else []):
                cc = _ceil(n_pad, 128)
                msg = mp.tile([128, CALLN // 128, HID], f32, tag="msg")
                nc.gpsimd.dma_gather(
                    out_ap=msg[:, :cc, :],
                    in_ap=table[w * WINSZ:(w + 1) * WINSZ, :],
                    idxs_ap=gidxA[:, off:off + n_pad // 16],
                    num_idxs=n_pad, num_idxs_reg=reg_of(n_pad), elem_size=HID,
                    single_packet=False, queue_num=k % 4)
                yo = mp.tile([128, CALLN // 128, 4], f32, tag="yo")
                nc.vector.tensor_copy(out=yo[:, :cc, :], in_=msg[:, :cc, 0:4])
                nc.sync.dma_start(
                    out=yout[soff:soff + cc * 128, :].rearrange(
                        "(p c) k -> p c k", p=128),
                    in_=yo[:, :cc, :])
                off += n_pad // 16
                soff += cc * 128
            if not body:
                yo0 = mp.tile([128, CALLN // 128, 4], f32, tag="yo")
                nc.vector.memset(yo0[:], 0.0)
                nc.sync.dma_start(
                    out=yout[0:(CALLN // 128) * 128, :].rearrange(
                        "(p c) k -> p c k", p=128),
                    in_=yo0[:, :, :])
    nc.compile()
    return nc, TOTS


def _emit_hw_blocks(nc, psA, xeT, convw, cbase, staging, dinvnm):
    for j in range(COLS):
        ps = psA.tile([128, HID], f32, tag="hw")
        _emit_block_mm(nc, ps, xeT, convw, cbase, j, out_w=HID)
        nc.vector.tensor_scalar_mul(out=staging[:, j, :], in0=ps[:],
                                    scalar1=dinvnm[:, j:j + 1])


def _emit_block_mm(nc, ps, xeT, rhs_tile, cbase, j, out_w):
    """ps[p, :] = (xe @ W) for node block j; rhs_tile is [128, *] with both
    halves stacked (rows 0:64 and 64:128 hold the same weights)."""
    fb = 64 * j
    nc.tensor.matmul(out=ps[0:64, :out_w], lhsT=xeT[0:64, fb:fb + 64],
                     rhs=rhs_tile[0:64, cbase:cbase + out_w],
                     start=True, stop=True)
    nc.tensor.matmul(out=ps[64:128, :out_w], lhsT=xeT[64:128, fb:fb + 64],
                     rhs=rhs_tile[64:128, cbase:cbase + out_w],
                     start=True, stop=True)


def _pack_inputs(inputs, dinv_g, cores, plan, fpacked):
    x = np.asarray(inputs["x"], np.float32)
    W_emb = np.asarray(inputs["W_emb"], np.float32)
    b_emb = np.asarray(inputs["b_emb"], np.float32)
    conv_W = np.asarray(inputs["conv_W"], np.float32)
    bn_gamma = np.asarray(inputs["bn_gamma"], np.float32)
    bn_beta = np.asarray(inputs["bn_beta"], np.float32)
    fc_W = np.asarray(inputs["fc_W"], np.float32)

    convw = np.transpose(conv_W, (1, 0, 2)).reshape(HID, L * HID)
    convw2 = np.concatenate([convw, convw], axis=0)          # [128, L*64]
    fcw_cat = np.concatenate([fc_W[:HID], fc_W[HID:]], axis=1)  # [64, 4]
    fcw2 = np.concatenate([fcw_cat, fcw_cat], axis=0)        # [128, 4]
    bn_cat = np.concatenate([bn_gamma, bn_beta], axis=1)     # [L, 128]
    bembT = np.tile(b_emb, 2).reshape(128, 1)

    NBLK = len(plan["blocks"])
    in_maps = []
    for c in range(NCORES):
        core = cores[c]
        xs = np.zeros((NPC, IN_DIM), np.float32)
        xs[:NPC_REAL] = x[c * NPC_REAL:(c + 1) * NPC_REAL]
        dv = np.zeros(NPC, np.float32)
        dv[:NPC_REAL] = dinv_g[c * NPC_REAL:(c + 1) * NPC_REAL]
        # xT[i, u, f] = x[slot(u,f), i], slot = 64*(f//32) + 32*u + f%32
        f = np.arange(HALFS)
        slot_u0 = 64 * (f // 32) + (f % 32)
        slot_u1 = slot_u0 + 32
        xT = np.stack([xs[slot_u0].T, xs[slot_u1].T], axis=1)  # [16, 2, HALFS]
        dinvnm = dv[SLOT_OF_PJ]                                # [128, COLS]
        dinvblk = np.zeros((128, 16 * NBLK), np.float32)
        for buid, arr in enumerate(core["dinvcols"]):
            dinvblk[:, 16 * buid:16 * buid + arr.shape[1]] = arr
        in_maps.append(dict(
            xT=np.ascontiguousarray(xT),
            dinvnm=np.ascontiguousarray(dinvnm),
            bembT=bembT, wemb=W_emb, convw=np.ascontiguousarray(convw2),
            bn=bn_cat, fcw=fcw2,
            dinvblk=dinvblk,
            gidxA=_wrap16(core["aidx"]),
            gidxB=_wrap16(core["bidx"]),
            fidx=np.concatenate([_wrap16(a) for a in fpacked[c]], axis=1),
        ))
    return in_maps


def _prepare(inputs):
    edge_index = np.asarray(inputs["edge_index"])
    edge_index_out = np.asarray(inputs["edge_index_out"])
    key = hash((edge_index[0, :50].tobytes(), edge_index_out[0, :50].tobytes()))
    if key in _CACHE:
        return _CACHE[key]
    dinv_g, plan, cores, fcalls, fpacked, fslotmap = _preprocess(
        edge_index, edge_index_out)
    nc, TOTS = _build_program(plan, fcalls)
    _CACHE[key] = (dinv_g, plan, cores, fcalls, fpacked, fslotmap, nc, TOTS)
    return _CACHE[key]


def kernel(x, edge_index, edge_index_out, W_emb, b_emb, conv_W, conv_b,
           bn_gamma, bn_beta, fc_W, fc_b):
    (dinv_g, plan, cores, fcalls, fpacked, fslotmap, nc, TOTS) = _prepare(
        dict(edge_index=edge_index, edge_index_out=edge_index_out))
    in_maps = _pack_inputs(
        dict(x=x, W_emb=W_emb, b_emb=b_emb, conv_W=conv_W, bn_gamma=bn_gamma,
             bn_beta=bn_beta, fc_W=fc_W),
        dinv_g, cores, plan, fpacked)
    res = run_bass_kernel_spmd(nc, in_maps, core_ids=list(range(NCORES)))

    EPC = E_OUT // NCORES
    out = np.zeros((E_OUT, OUT_DIM), np.float32)
    fc_b = np.asarray(fc_b, np.float32)
    for c in range(NCORES):
        y = res.results[c]["yout"]
        soff = 0
        for k, (w, n_pad) in enumerate(fcalls):
            cc = _ceil(n_pad, 128)
            eid, half = fslotmap[c][k]
            nreal = len(eid)
            i = np.arange(nreal)
            rows = soff + (i % 128) * cc + i // 128
            vals = y[rows]
            sel_src = half == 0
            out[c * EPC + eid[sel_src], :] += vals[sel_src][:, 0:2]
            out[c * EPC + eid[~sel_src], :] += vals[~sel_src][:, 2:4]
            soff += cc * 128
    out += fc_b[None, :]
    return out
